# revision 1
# baseline (speedup 1.0000x reference)
"""Multi-head attention (b=4, n=2048, dim=1024, 16 heads x 64) on 8 Trainium2
NeuronCores.

Sharding: data-parallel over batch (4) x tensor-parallel over head-groups (2).
Each core gets one batch element and 8 heads; the host sums the two
head-group partials per batch element and adds b_out.

Per-core pipeline (fp32r proj / bf16 attention operands):
  A:  x^T arrives pre-transposed + pre-cast to bf16 from the host; [128, 512]
      slices are DMA'd straight into SBUF (no PE transposes).  qT / kT strips
      ([inner, n], bf16) and v (natural [n, inner] + a ones column per head,
      bf16) stay SBUF-resident -- no DRAM staging.  Phase A's projection
      strips are WOVEN into phase B's step stream (each strip emitted just
      before its first consuming attention step), so the PE fills ScalarE's
      pacing slack with projection work instead of idling for 100us.
  B:  i-blocks outer, heads inner: S^T j-tiles = matmul(lhsT=k^T_h j-block,
      rhs=q^T_h i-block); exp on ScalarE (1/sqrt(dh) folded into the
      activation scale) writes bf16 pexp; PV accumulates O_aug^T in PSUM
      ([dh+1, 512]; last row = softmax denominator), trailing exp by
      pend_depth steps so the PE never stalls on ScalarE's exp latency.
      Tail: DVE stages po to SBUF + tiny reciprocal; GPSIMD broadcasts the
      reciprocal row and multiplies into oT (GPSIMD has no PSUM port, so
      it only ever touches SBUF).
  C:  y = O @ w_out via lhsT = oT strips (fp32r), micro-woven (matmul at a
      time) into attention steps where the phase-A stream has nothing due;
      DVE drains the proj psums, y DMA'd out per 512-column chunk.
"""

from contextlib import ExitStack

import numpy as np
import ml_dtypes

import concourse.mybir as mybir
import concourse.tile as tile
from concourse import bacc, bass_utils

F32 = mybir.dt.float32
F32R = mybir.dt.float32r
BF16 = mybir.dt.bfloat16
AF = mybir.ActivationFunctionType
ALU = mybir.AluOpType

# Full-problem constants (hardcoded per the harness contract).
B_FULL, N_FULL, DIM_FULL = 4, 2048, 1024
HEADS_FULL, DH = 16, 64
N_CORES = 8
GROUPS = 2                       # head-group (tensor-parallel) factor
HPC = HEADS_FULL // GROUPS       # heads per core = 8
INNER_PC = HPC * DH              # per-core inner dim = 512


def ts(i, size):
    return slice(i * size, (i + 1) * size)


def emit_core_kernel(nc, tc, xt, wq, wk, wv, wout, y, *, n, dim, hpc,
                     ib=1024, proj_slack=6, ahead=0, ou_eng="gps",
                     proj_every=2, pend_depth=7, pexp_bufs=8):
    dh = DH
    inner = hpc * dh
    KC = dim // 128          # contraction chunks
    S = inner // 128         # 128-row strips of the inner dim
    JT = n // 128            # key/value j-tiles
    NB = n // 512            # 512-wide n-chunks in phase A
    ib = min(ib, n)
    n_ibx = n // ib
    itpb = ib // 128
    cpb = ib // 512          # 512-chunks per i-block
    scale = float(1.0 / np.sqrt(dh))
    fc = min(512, dim)
    assert n % 512 == 0 and dim % 128 == 0 and ib % 512 == 0

    stack = ExitStack()
    with stack:
        persist = stack.enter_context(tc.tile_pool(name="persist", bufs=1))
        w_pool = stack.enter_context(tc.tile_pool(name="a_w", bufs=1))
        xts_pool = stack.enter_context(
            tc.tile_pool(name="a_xts", bufs=KC * NB + 2))
        psA_pool = stack.enter_context(
            tc.tile_pool(name="a_ps", bufs=2, space="PSUM"))

        qT = [persist.tile([128, n], BF16, name="qT", tag=f"qT{s}")
              for s in range(S)]
        kT = [persist.tile([128, n], BF16, name="kT", tag=f"kT{s}")
              for s in range(S)]
        v_sb = [persist.tile([128, hpc * (dh + 1)], BF16, name="vt",
                             tag=f"v{j}") for j in range(JT)]
        for j in range(JT):
            nc.gpsimd.memset(
                v_sb[j].rearrange("p (h c) -> p h c", c=dh + 1)
                [:, :, dh:dh + 1], 1.0)
        oT = [persist.tile([128, n], F32R, name="oT", tag=f"oT{s}")
              for s in range(S)]

        # weight DMAs: wk first (first strips), wv/wq next, wout last (only
        # needed by the first proj group, ~100us in).
        wq_sb, wk_sb, wv_sb, wout_sb = [], [], [], []
        xts_tiles = {}

        def xts(c, nb, eng=None):
            t = xts_tiles.get((c, nb))
            if t is None:
                t = xts_pool.tile([128, 512], BF16, name="xts")
                if eng is None:
                    eng = nc.gpsimd if (c % 2) == 1 else nc.sync
                eng.dma_start(t, xt[c][:, ts(nb, 512)])
                xts_tiles[(c, nb)] = t
            return t

        # DMA priority order: strip-0 columns of wk (+x chunk 0) feed the
        # very first projection strips; then strip-0 of wq (+x chunk 1),
        # then wv (first v tile ~10 steps in), then the remaining w columns,
        # remaining x chunks, and wout (first consumed ~100us in) last.
        for nm, lst in (("wk", wk_sb), ("wq", wq_sb)):
            if inner > 128:
                for c in range(KC):
                    lst.append(w_pool.tile([128, inner - 128], BF16,
                                           name="wt", tag=f"{nm}{c}"))
        for c in range(KC):
            wv_sb.append(w_pool.tile([128, inner], BF16, name="wt",
                                     tag=f"wv{c}"))
        # strip-0 weight columns for all chunks land as ONE strided DMA
        # each (the per-DMA queue cadence, not transfer size, paces startup)
        wks0 = w_pool.tile([128, KC * 128], BF16, name="wks0")
        nc.sync.dma_start(wks0.rearrange("p (c w) -> p c w", c=KC),
                          wk[:, :, 0:128].rearrange("c p w -> p c w"))
        for c in range(KC):
            xts(c, 0)
        wqs0 = w_pool.tile([128, KC * 128], BF16, name="wqs0")
        nc.sync.dma_start(wqs0.rearrange("p (c w) -> p c w", c=KC),
                          wq[:, :, 0:128].rearrange("c p w -> p c w"))
        if NB > 1:
            for c in range(KC):
                xts(c, 1)
        for c in range(KC):
            nc.sync.dma_start(wv_sb[c], wv[c])
        for c in range(KC):
            if inner > 128:
                nc.sync.dma_start(wk_sb[c], wk[c][:, 128:inner])
                nc.sync.dma_start(wq_sb[c], wq[c][:, 128:inner])
        for nb in range(2, NB):
            for c in range(KC):
                xts(c, nb)
        for t in range(S):
            wo = persist.tile([128, dim], F32R, name="wo", tag=f"wo{t}")
            nc.sync.dma_start(wo, wout[t])
            wout_sb.append(wo)


        # ---- phase A ops (generators, woven into phase B at matmul
        #      granularity) ----
        def qk_strip_gen(w_sb, dst, t, nb):
            s0 = wks0 if w_sb is wk_sb else wqs0
            ps = psA_pool.tile([128, 512], F32, name="psA")
            for c in range(KC):
                lhsT = (s0[:, ts(c, 128)] if t == 0
                        else w_sb[c][:, ts(t - 1, 128)])
                nc.tensor.matmul(ps, lhsT, xts(c, nb),
                                 start=(c == 0), stop=(c == KC - 1))
                yield
            nc.vector.tensor_copy(dst[t][:, ts(nb, 512)], ps)

        def v_tile_gen(it):
            ps = psA_pool.tile([128, inner], F32, name="psA")
            for c in range(KC):
                nc.tensor.matmul(ps, xts(c, it // 4)[:, ts(it % 4, 128)],
                                 wv_sb[c], start=(c == 0), stop=(c == KC - 1))
                yield
            nc.vector.tensor_copy(
                v_sb[it].rearrange("p (h c) -> p h c", c=dh + 1)[:, :, 0:dh],
                ps.rearrange("p (h d) -> p h d", d=dh))

        # need-index: flat B step index (ibx*hpc*JT + h*JT + jt) of the first
        # consumer of each A op.
        a_ops = []
        for s in range(S):
            for nb in range(NB):
                a_ops.append((2 * s * JT + 4 * nb, 0,
                              lambda s=s, nb=nb: qk_strip_gen(
                                  wk_sb, kT, s, nb)))
        for s in range(S):
            for ibx in range(n_ibx):
                for nb in range(ibx * ib // 512, (ibx + 1) * ib // 512):
                    a_ops.append((ibx * hpc * JT + 2 * s * JT, 1,
                                  lambda s=s, nb=nb: qk_strip_gen(
                                      wq_sb, qT, s, nb)))
        for it in range(JT):
            a_ops.append((it + 1, 2, lambda it=it: v_tile_gen(it)))
        a_ops.sort(key=lambda x: (x[0], x[1]))
        a_state = {"ptr": 0, "gen": None}

        def a_step():
            """Advance the A stream one micro-op; False when exhausted."""
            while True:
                if a_state["gen"] is None:
                    if a_state["ptr"] >= len(a_ops):
                        return False
                    a_state["gen"] = a_ops[a_state["ptr"]][2]()
                try:
                    next(a_state["gen"])
                    return True
                except StopIteration:
                    a_state["gen"] = None
                    a_state["ptr"] += 1

        def pump_a(limit, budget=None):
            n_done = 0
            while True:
                if budget is not None and n_done >= budget:
                    return
                if a_state["ptr"] >= len(a_ops):
                    return
                if budget is None and a_ops[a_state["ptr"]][0] > limit:
                    return
                if not a_step():
                    return
                n_done += 1

        # ---- phase B/C ----
        with (
            tc.tile_pool(name="b_psS", bufs=2, space="PSUM") as psS_pool,
            tc.tile_pool(name="b_psO", bufs=2, space="PSUM") as psO_pool,
            tc.tile_pool(name="b_pexp", bufs=pexp_bufs) as pexp_pool,
            tc.tile_pool(name="b_ou", bufs=4) as ou_pool,
            tc.tile_pool(name="b_bc", bufs=4) as bc_pool,
            tc.tile_pool(name="c_y", bufs=3) as y_pool,
        ):
            ysb_open = {}

            def emit_proj_group(it, cc, final=False):
                if cc == 0:
                    ysb_open[it] = y_pool.tile([128, dim], F32, name="ysb")
                ysb = ysb_open[it]
                if final:
                    # alternate the (by now idle) psS slots with psA so the
                    # last i-block's groups pipeline 4 deep
                    if (it * (dim // fc) + cc) % 2 == 0:
                        ps = psS_pool.tile([128, fc], F32, name="psS")
                    else:
                        ps = psA_pool.tile([128, fc], F32, name="psA")
                else:
                    ps = psA_pool.tile([128, fc], F32, name="psA")
                for t in range(S):
                    nc.tensor.matmul(
                        ps, oT[t][:, ts(it, 128)], wout_sb[t][:, ts(cc, fc)],
                        start=(t == 0), stop=(t == S - 1))
                nc.vector.tensor_copy(ysb[:, ts(cc, fc)], ps)
                nc.sync.dma_start(y[ts(it, 128), ts(cc, fc)],
                                  ysb[:, ts(cc, fc)])
                if cc == dim // fc - 1:
                    del ysb_open[it]

            proj_due = []
            proj_state = {"gen": None}
            pend = []             # one deferred PV (+ tail) step
            po_live = {}

            def proj_group_gen(it, cc):
                if cc == 0:
                    ysb_open[it] = y_pool.tile([128, dim], F32, name="ysb")
                ysb = ysb_open[it]
                ps = psA_pool.tile([128, fc], F32, name="psA")
                for t in range(S):
                    nc.tensor.matmul(
                        ps, oT[t][:, ts(it, 128)], wout_sb[t][:, ts(cc, fc)],
                        start=(t == 0), stop=(t == S - 1))
                    yield
                nc.vector.tensor_copy(ysb[:, ts(cc, fc)], ps)
                nc.sync.dma_start(y[ts(it, 128), ts(cc, fc)],
                                  ysb[:, ts(cc, fc)])
                if cc == dim // fc - 1:
                    del ysb_open[it]

            def pump_proj(budget):
                n_done = 0
                while n_done < budget and (proj_state["gen"] or proj_due):
                    if proj_state["gen"] is None:
                        proj_state["gen"] = proj_group_gen(*proj_due.pop(0))
                    try:
                        next(proj_state["gen"])
                        n_done += 1
                    except StopIteration:
                        proj_state["gen"] = None

            def a_idle(idx):
                return (a_state["ptr"] >= len(a_ops)
                        and a_state["gen"] is None) or (
                    a_state["gen"] is None
                    and a_ops[a_state["ptr"]][0] > idx + ahead)

            def emit_pv(h, ibx, jt, pexp):
                po = po_live[(h, ibx)]
                s_, r_ = divmod(h * dh, 128)
                vcol = slice(h * (dh + 1), (h + 1) * (dh + 1))
                for cc in range(cpb):
                    nc.tensor.matmul(
                        po[cc], v_sb[jt][:, vcol], pexp[:, ts(cc, 512)],
                        start=(jt == 0), stop=(jt == JT - 1))
                if jt == JT - 1:
                    po_live.pop((h, ibx))
                    last_head = (ibx == n_ibx - 1 and h == hpc - 1)
                    for cc in range(cpb):
                        off = ibx * ib + cc * 512
                        if ou_eng == "gps" and not last_head:
                            # stage po to SBUF (frees the psum bank early),
                            # then GPSIMD does broadcast + multiply so DVE
                            # only pays one copy + one tiny reciprocal
                            ou = ou_pool.tile([dh + 1, 512], F32, name="ou")
                            nc.vector.tensor_copy(ou, po[cc])
                            recip = ou_pool.tile([1, 512], F32, name="recip")
                            nc.vector.reciprocal(recip, ou[dh:dh + 1, :])
                            bc = bc_pool.tile([dh, 512], F32, name="bc")
                            nc.gpsimd.partition_broadcast(bc, recip)
                            nc.gpsimd.tensor_tensor(
                                oT[s_][r_:r_ + dh, off:off + 512],
                                ou[0:dh, :], bc, ALU.mult)
                        else:
                            recip = ou_pool.tile([1, 512], F32, name="recip")
                            nc.vector.reciprocal(recip, po[cc][dh:dh + 1, :])
                            bc = bc_pool.tile([dh, 512], F32, name="bc")
                            nc.gpsimd.partition_broadcast(bc, recip)
                            nc.vector.tensor_mul(
                                oT[s_][r_:r_ + dh, off:off + 512],
                                po[cc][0:dh, :], bc)

            for ibx in range(n_ibx):
                for h in range(hpc):
                    po_live[(h, ibx)] = [
                        psO_pool.tile([dh + 1, 512], F32, name="po")
                        for _ in range(cpb)]
                    s_, r_ = divmod(h * dh, 128)
                    kTh = kT[s_][r_:r_ + dh, :]
                    qTh = qT[s_][r_:r_ + dh, :]
                    for jt in range(JT):
                        idx = ibx * hpc * JT + h * JT + jt
                        pump_a(idx)
                        psS = psS_pool.tile([128, ib], F32, name="psS")
                        for cc in range(cpb):
                            nc.tensor.matmul(
                                psS[:, ts(cc, 512)], kTh[:, ts(jt, 128)],
                                qTh[:, ibx * ib + cc * 512:
                                    ibx * ib + (cc + 1) * 512],
                                start=True, stop=True)
                        pexp = pexp_pool.tile([128, ib], BF16, name="pexp")
                        nc.scalar.activation(pexp, psS, AF.Exp, scale=scale)
                        pend.append((h, ibx, jt, pexp))
                        while len(pend) > pend_depth:
                            emit_pv(*pend.pop(0))
                        pump_a(idx + ahead)
                        if a_idle(idx):
                            pump_proj(proj_every)
                pump_proj(10 ** 9)
                proj_due = [(it, cc)
                            for it in range(ibx * itpb, (ibx + 1) * itpb)
                            for cc in range(dim // fc)]
                if ibx == n_ibx - 1:
                    while pend:
                        emit_pv(*pend.pop(0))
                    for it, cc in proj_due:
                        emit_proj_group(it, cc, final=True)
                    proj_due = []
            pump_a(10 ** 9)


_BUILD_CACHE = {}


def build_nc(n=N_FULL, dim=DIM_FULL, hpc=HPC, ib=1024, proj_slack=6,
             ahead=0, ou_eng="gps", proj_every=2, pend_depth=7,
             pexp_bufs=8):
    key = (n, dim, hpc, ib, proj_slack, ahead, ou_eng, proj_every,
           pend_depth, pexp_bufs)
    if key in _BUILD_CACHE:
        return _BUILD_CACHE[key]
    inner = hpc * DH
    KC = dim // 128
    S = inner // 128
    nc = bacc.Bacc("TRN2", target_bir_lowering=False, debug=False)
    xt = nc.dram_tensor("xt", [KC, 128, n], BF16, kind="ExternalInput").ap()
    wq = nc.dram_tensor("wq", [KC, 128, inner], BF16,
                        kind="ExternalInput").ap()
    wk = nc.dram_tensor("wk", [KC, 128, inner], BF16,
                        kind="ExternalInput").ap()
    wv = nc.dram_tensor("wv", [KC, 128, inner], BF16,
                        kind="ExternalInput").ap()
    wout = nc.dram_tensor("wout", [S, 128, dim], F32R,
                          kind="ExternalInput").ap()
    y = nc.dram_tensor("y", [n, dim], F32, kind="ExternalOutput").ap()
    with tile.TileContext(nc) as tc:
        with nc.allow_low_precision(
                reason="bf16 attention operands; fp32 accumulation"):
            emit_core_kernel(nc, tc, xt, wq, wk, wv, wout, y, n=n, dim=dim,
                             hpc=hpc, ib=ib, proj_slack=proj_slack,
                             ahead=ahead, ou_eng=ou_eng,
                             proj_every=proj_every, pend_depth=pend_depth,
                             pexp_bufs=pexp_bufs)
    nc.compile()
    _BUILD_CACHE[key] = nc
    return nc


def prep_core_inputs(x, w_qkv, w_out, n, dim, hpc):
    """Host-side prep for ONE core (layout only: transpose/chunk/cast)."""
    inner = hpc * DH
    KC = dim // 128
    x = np.asarray(x, np.float32)
    w_qkv = np.asarray(w_qkv, np.float32)
    bf = ml_dtypes.bfloat16
    xt = np.ascontiguousarray(x.T).reshape(KC, 128, n).astype(bf)
    return {
        "xt": xt,
        "wq": np.ascontiguousarray(
            w_qkv[:, 0:inner].reshape(KC, 128, inner)).astype(bf),
        "wk": np.ascontiguousarray(
            w_qkv[:, inner:2 * inner].reshape(KC, 128, inner)).astype(bf),
        "wv": np.ascontiguousarray(
            w_qkv[:, 2 * inner:3 * inner].reshape(KC, 128, inner)).astype(bf),
        "wout": np.ascontiguousarray(
            np.asarray(w_out, np.float32).reshape(inner // 128, 128, dim)),
    }


def make_in_maps(x, w_qkv, w_out):
    """Shard full inputs into the 8 per-core input maps."""
    x = np.asarray(x, dtype=np.float32)
    w_qkv = np.asarray(w_qkv, dtype=np.float32)
    w_out = np.asarray(w_out, dtype=np.float32)
    qk = HEADS_FULL * DH
    in_maps = []
    for core in range(N_CORES):
        b, g = divmod(core, GROUPS)
        cols = ts(g, INNER_PC)
        wqkv_c = np.concatenate(
            [w_qkv[:, cols],
             w_qkv[:, qk + g * INNER_PC:qk + (g + 1) * INNER_PC],
             w_qkv[:, 2 * qk + g * INNER_PC:2 * qk + (g + 1) * INNER_PC]],
            axis=1)
        in_maps.append(prep_core_inputs(x[b], wqkv_c, w_out[cols, :],
                                        N_FULL, DIM_FULL, HPC))
    return in_maps


def kernel(x, w_qkv, w_out, b_out, trace=False):
    b_out = np.asarray(b_out, dtype=np.float32)
    in_maps = make_in_maps(x, w_qkv, w_out)
    nc = build_nc()
    res = bass_utils.run_bass_kernel_spmd(
        nc, in_maps, core_ids=list(range(N_CORES)), trace=trace)
    ys = [r["y"] for r in res.results]
    out = np.empty((B_FULL, N_FULL, DIM_FULL), dtype=np.float32)
    for b in range(B_FULL):
        out[b] = ys[GROUPS * b] + ys[GROUPS * b + 1] + b_out[None, :]
    if trace:
        kernel.last_result = res
    return out



# revision 12
# speedup vs baseline: 1.0489x; 1.0489x over previous
"""Multi-head attention (b=4, n=2048, dim=1024, 16 heads x 64) on 8 Trainium2
NeuronCores.

Sharding: data-parallel over batch (4) x tensor-parallel over head-groups (2).
Each core gets one batch element and 8 heads; the host sums the two
head-group partials per batch element and adds b_out.

Per-core pipeline (bf16 operands everywhere; fp32 accumulation):
  A:  x^T arrives pre-transposed + pre-cast to bf16 from the host; [128, 512]
      slices are DMA'd straight into SBUF (no PE transposes).  qT / kT strips
      ([inner, n], bf16) and v (natural [n, inner] + a ones column per head,
      bf16) stay SBUF-resident.  Phase A's projection strips are WOVEN into
      phase B's step stream (each strip emitted just before its first
      consuming attention step), so the PE fills the startup DMA window with
      projection work instead of idling.
  B:  per (i-block, head): S^T j-tiles = matmul(lhsT=k^T_h j-block,
      rhs=q^T_h i-block); exp on ScalarE (1/sqrt(dh) folded into the
      activation scale) writes bf16 pexp tiles which stay resident for the
      whole head.  PV is FLIPPED: po[i-128, dh+1] psum accumulates
      matmul(lhsT=pexp_jt[:, i-chunk], rhs=v_jt[:, head cols]) over all 16
      j-tiles -- out free size is only dh+1=65, so PV costs ~2x fewer PE
      cycles than the [dh+1, i]-oriented version (cost model charges
      out-free-size per pass).  The last v column is ones, so po[:, dh] is
      the softmax denominator: DVE reciprocal + per-partition tensor_scalar
      multiply normalizes straight into a bf16 staging tile, PE transposes
      it (identity trick) to [dh, 128] psum, DVE copies into the oT strips.
      PV/normalize/transpose of head h-1 is interleaved into head h's score
      stream so each small psum pool (1 buf each) has time to drain.
  C:  y = O @ w_out via lhsT = oT strips (bf16), micro-woven (matmul at a
      time) into attention steps where the phase-A stream has nothing due;
      DVE drains the proj psums, y DMA'd out per 512-column chunk.
"""

from contextlib import ExitStack

import numpy as np
import ml_dtypes

import concourse.mybir as mybir
import concourse.tile as tile
from concourse import bacc, bass_utils, masks

F32 = mybir.dt.float32
BF16 = mybir.dt.bfloat16
AF = mybir.ActivationFunctionType
ALU = mybir.AluOpType

# Full-problem constants (hardcoded per the harness contract).
B_FULL, N_FULL, DIM_FULL = 4, 2048, 1024
HEADS_FULL, DH = 16, 64
N_CORES = 8
GROUPS = 2                       # head-group (tensor-parallel) factor
HPC = HEADS_FULL // GROUPS       # heads per core = 8
INNER_PC = HPC * DH              # per-core inner dim = 512


def ts(i, size):
    return slice(i * size, (i + 1) * size)


def emit_core_kernel(nc, tc, xt, wq, wk, wv, wout, y, *, n, dim, hpc,
                     ib=1024, ahead=0, proj_every=2):
    dh = DH
    inner = hpc * dh
    KC = dim // 128          # contraction chunks
    S = inner // 128         # 128-row strips of the inner dim
    JT = n // 128            # key/value j-tiles
    NB = n // 512            # 512-wide n-chunks in phase A
    ib = min(ib, n)
    n_ibx = n // ib
    itpb = ib // 128         # i-128 chunks per i-block
    cpb = ib // 512          # 512-chunks per i-block
    scale = float(1.0 / np.sqrt(dh))
    fc = min(512, dim)
    assert n % 512 == 0 and dim % 128 == 0 and ib % 512 == 0

    stack = ExitStack()
    with stack:
        persist = stack.enter_context(tc.tile_pool(name="persist", bufs=1))
        w_pool = stack.enter_context(tc.tile_pool(name="a_w", bufs=1))
        xts_pool = stack.enter_context(
            tc.tile_pool(name="a_xts", bufs=KC * NB + 2))
        psA_pool = stack.enter_context(
            tc.tile_pool(name="a_ps", bufs=2, space="PSUM"))

        qT = [persist.tile([128, n], BF16, name="qT", tag=f"qT{s}")
              for s in range(S)]
        kT = [persist.tile([128, n], BF16, name="kT", tag=f"kT{s}")
              for s in range(S)]
        v_sb = [persist.tile([128, hpc * (dh + 1)], BF16, name="vt",
                             tag=f"v{j}") for j in range(JT)]
        for j in range(JT):
            nc.gpsimd.memset(
                v_sb[j].rearrange("p (h c) -> p h c", c=dh + 1)
                [:, :, dh:dh + 1], 1.0)
        oT = [persist.tile([128, n], BF16, name="oT", tag=f"oT{s}")
              for s in range(S)]
        ident = persist.tile([128, 128], BF16, name="ident")
        masks.make_identity(nc, ident)

        # weight DMAs: wk first (first strips), wv/wq next, wout last (only
        # needed by the first proj group, far in).
        wq_sb, wk_sb, wv_sb, wout_sb = [], [], [], []
        xts_tiles = {}

        def xts(c, nb, eng=None):
            t = xts_tiles.get((c, nb))
            if t is None:
                t = xts_pool.tile([128, 512], BF16, name="xts")
                if eng is None:
                    eng = nc.gpsimd if (c % 2) == 1 else nc.sync
                eng.dma_start(t, xt[c][:, ts(nb, 512)])
                xts_tiles[(c, nb)] = t
            return t

        # DMA priority order: strip-0 columns of wk (+x chunk 0) feed the
        # very first projection strips; then strip-0 of wq (+x chunk 1),
        # then wv (first v tile ~10 steps in), then the remaining w columns,
        # remaining x chunks, and wout (first consumed much later) last.
        for nm, lst in (("wk", wk_sb), ("wq", wq_sb)):
            if inner > 128:
                for c in range(KC):
                    lst.append(w_pool.tile([128, inner - 128], BF16,
                                           name="wt", tag=f"{nm}{c}"))
        for c in range(KC):
            wv_sb.append(w_pool.tile([128, inner], BF16, name="wt",
                                     tag=f"wv{c}"))
        # strip-0 weight columns for all chunks land as ONE strided DMA
        # each (the per-DMA queue cadence, not transfer size, paces startup)
        wks0 = w_pool.tile([128, KC * 128], BF16, name="wks0")
        nc.sync.dma_start(wks0.rearrange("p (c w) -> p c w", c=KC),
                          wk[:, :, 0:128].rearrange("c p w -> p c w"))
        for c in range(KC):
            xts(c, 0)
        wqs0 = w_pool.tile([128, KC * 128], BF16, name="wqs0")
        nc.sync.dma_start(wqs0.rearrange("p (c w) -> p c w", c=KC),
                          wq[:, :, 0:128].rearrange("c p w -> p c w"))
        if NB > 1:
            for c in range(KC):
                xts(c, 1)
        for c in range(KC):
            nc.sync.dma_start(wv_sb[c], wv[c])
        for c in range(KC):
            if inner > 128:
                nc.sync.dma_start(wk_sb[c], wk[c][:, 128:inner])
                nc.sync.dma_start(wq_sb[c], wq[c][:, 128:inner])
        for nb in range(2, NB):
            for c in range(KC):
                xts(c, nb)
        for t in range(S):
            wo = persist.tile([128, dim], BF16, name="wo", tag=f"wo{t}")
            nc.sync.dma_start(wo, wout[t])
            wout_sb.append(wo)

        # ---- phase A ops (generators, woven into phase B at matmul
        #      granularity) ----
        def qk_strip_gen(w_sb, dst, t, nb):
            s0 = wks0 if w_sb is wk_sb else wqs0
            ps = psA_pool.tile([128, 512], F32, name="psA")
            for c in range(KC):
                lhsT = (s0[:, ts(c, 128)] if t == 0
                        else w_sb[c][:, ts(t - 1, 128)])
                nc.tensor.matmul(ps, lhsT, xts(c, nb),
                                 start=(c == 0), stop=(c == KC - 1))
                yield
            nc.vector.tensor_copy(dst[t][:, ts(nb, 512)], ps)

        def v_tile_gen(it):
            ps = psA_pool.tile([128, inner], F32, name="psA")
            for c in range(KC):
                nc.tensor.matmul(ps, xts(c, it // 4)[:, ts(it % 4, 128)],
                                 wv_sb[c], start=(c == 0), stop=(c == KC - 1))
                yield
            nc.vector.tensor_copy(
                v_sb[it].rearrange("p (h c) -> p h c", c=dh + 1)[:, :, 0:dh],
                ps.rearrange("p (h d) -> p h d", d=dh))

        # need-index: flat B step index (ibx*hpc*JT + h*JT + jt) of the first
        # consumer of each A op.
        a_ops = []
        for s in range(S):
            for nb in range(NB):
                a_ops.append((2 * s * JT + 4 * nb, 0,
                              lambda s=s, nb=nb: qk_strip_gen(
                                  wk_sb, kT, s, nb)))
        for s in range(S):
            for ibx in range(n_ibx):
                for nb in range(ibx * ib // 512, (ibx + 1) * ib // 512):
                    a_ops.append((ibx * hpc * JT + 2 * s * JT, 1,
                                  lambda s=s, nb=nb: qk_strip_gen(
                                      wq_sb, qT, s, nb)))
        for it in range(JT):
            a_ops.append((it + 1, 2, lambda it=it: v_tile_gen(it)))
        a_ops.sort(key=lambda x: (x[0], x[1]))
        a_state = {"ptr": 0, "gen": None}

        def a_step():
            """Advance the A stream one micro-op; False when exhausted."""
            while True:
                if a_state["gen"] is None:
                    if a_state["ptr"] >= len(a_ops):
                        return False
                    a_state["gen"] = a_ops[a_state["ptr"]][2]()
                try:
                    next(a_state["gen"])
                    return True
                except StopIteration:
                    a_state["gen"] = None
                    a_state["ptr"] += 1

        def pump_a(limit, budget=None):
            n_done = 0
            while True:
                if budget is not None and n_done >= budget:
                    return
                if a_state["ptr"] >= len(a_ops):
                    return
                if budget is None and a_ops[a_state["ptr"]][0] > limit:
                    return
                if not a_step():
                    return
                n_done += 1

        # ---- phase B/C ----
        with (
            tc.tile_pool(name="b_psS", bufs=2, space="PSUM") as psS_pool,
            # po ([128, dh+1] f32) and trp ([dh, 128] bf16) alternate through
            # the same two bank-sized slots: po(ci+1) waits only on po(ci)'s
            # DVE readers, trp(ci+1) only on trp(ci)'s DVE copy
            tc.tile_pool(name="b_psO", bufs=2, space="PSUM") as psO_pool,
            tc.tile_pool(name="b_pexp", bufs=2 * JT) as pexp_pool,
            tc.tile_pool(name="b_ost", bufs=3) as ost_pool,
            tc.tile_pool(name="b_rec", bufs=3) as rec_pool,
            tc.tile_pool(name="c_y", bufs=2) as y_pool,
        ):
            ysb_open = {}

            def emit_proj_group(it, cc, final=False):
                if cc == 0:
                    ysb_open[it] = y_pool.tile([128, dim], F32, name="ysb")
                ysb = ysb_open[it]
                if final:
                    # alternate psA with the (by now idle) psS slots so the
                    # last i-block's groups pipeline 4 deep
                    if (it * (dim // fc) + cc) % 2 == 0:
                        ps = psS_pool.tile([128, fc], F32, name="psS")
                    else:
                        ps = psA_pool.tile([128, fc], F32, name="psA")
                else:
                    ps = psA_pool.tile([128, fc], F32, name="psA")
                for t in range(S):
                    nc.tensor.matmul(
                        ps, oT[t][:, ts(it, 128)], wout_sb[t][:, ts(cc, fc)],
                        start=(t == 0), stop=(t == S - 1))
                nc.vector.tensor_copy(ysb[:, ts(cc, fc)], ps)
                nc.sync.dma_start(y[ts(it, 128), ts(cc, fc)],
                                  ysb[:, ts(cc, fc)])
                if cc == dim // fc - 1:
                    del ysb_open[it]

            proj_due = []
            proj_state = {"gen": None}

            def proj_group_gen(it, cc):
                if cc == 0:
                    ysb_open[it] = y_pool.tile([128, dim], F32, name="ysb")
                ysb = ysb_open[it]
                ps = psA_pool.tile([128, fc], F32, name="psA")
                for t in range(S):
                    nc.tensor.matmul(
                        ps, oT[t][:, ts(it, 128)], wout_sb[t][:, ts(cc, fc)],
                        start=(t == 0), stop=(t == S - 1))
                    yield
                nc.vector.tensor_copy(ysb[:, ts(cc, fc)], ps)
                nc.sync.dma_start(y[ts(it, 128), ts(cc, fc)],
                                  ysb[:, ts(cc, fc)])
                if cc == dim // fc - 1:
                    del ysb_open[it]

            def pump_proj(budget):
                n_done = 0
                while n_done < budget and (proj_state["gen"] or proj_due):
                    if proj_state["gen"] is None:
                        proj_state["gen"] = proj_group_gen(*proj_due.pop(0))
                    try:
                        next(proj_state["gen"])
                        n_done += 1
                    except StopIteration:
                        proj_state["gen"] = None

            def a_idle(idx):
                return (a_state["ptr"] >= len(a_ops)
                        and a_state["gen"] is None) or (
                    a_state["gen"] is None
                    and a_ops[a_state["ptr"]][0] > idx + ahead)

            def pv_gen(h, ibx, pexp_tiles):
                """Flipped PV + normalize + transpose for one (head, i-block).
                One yield per i-128 chunk; each chunk's transpose+copy are
                deferred to the NEXT step so the PE never waits on the
                freshly-issued DVE recip/normalize chain."""
                s_, r_ = divmod(h * dh, 128)
                vcol = slice(h * (dh + 1), (h + 1) * (dh + 1))

                def finish(ost, ci):
                    c0 = ibx * ib + ci * 128
                    trp = psO_pool.tile([dh, 128], BF16, name="trp", tag="pot")
                    nc.tensor.transpose(trp, ost, ident)
                    nc.vector.tensor_copy(oT[s_][r_:r_ + dh, c0:c0 + 128],
                                          trp)

                prev = None
                for ci in range(itpb):
                    if prev is not None:
                        finish(*prev)
                    po = psO_pool.tile([128, dh + 1], F32, name="po", tag="pot")
                    for jt in range(JT):
                        nc.tensor.matmul(
                            po, pexp_tiles[jt][:, ts(ci, 128)],
                            v_sb[jt][:, vcol],
                            start=(jt == 0), stop=(jt == JT - 1))
                    recip = rec_pool.tile([128, 1], F32, name="recip")
                    nc.vector.reciprocal(recip, po[:, dh:dh + 1])
                    ost = ost_pool.tile([128, dh], BF16, name="ost")
                    nc.vector.tensor_scalar_mul(ost, po[:, 0:dh], recip)
                    prev = (ost, ci)
                    yield
                finish(*prev)

            pv_state = {"gen": None, "done": 0, "block_done": None}
            _DONE = object()

            def pump_pv(want):
                st = pv_state
                while st["gen"] is not None and st["done"] < want:
                    if next(st["gen"], _DONE) is _DONE:
                        st["gen"] = None
                        if st["block_done"] is not None:
                            # last head of block finished: its out-projection
                            # groups may now be emitted (all oT writes for the
                            # block precede them in program order)
                            bx = st["block_done"]
                            proj_due.extend(
                                (it, cc)
                                for it in range(bx * itpb, (bx + 1) * itpb)
                                for cc in range(dim // fc))
                        break
                    st["done"] += 1

            for ibx in range(n_ibx):
                for h in range(hpc):
                    s_, r_ = divmod(h * dh, 128)
                    kTh = kT[s_][r_:r_ + dh, :]
                    qTh = qT[s_][r_:r_ + dh, :]
                    pexp_tiles = []
                    for jt in range(JT):
                        idx = ibx * hpc * JT + h * JT + jt
                        pump_a(idx)
                        psS = psS_pool.tile([128, ib], F32, name="psS")
                        for cc in range(cpb):
                            nc.tensor.matmul(
                                psS[:, ts(cc, 512)], kTh[:, ts(jt, 128)],
                                qTh[:, ibx * ib + cc * 512:
                                    ibx * ib + (cc + 1) * 512],
                                start=True, stop=True)
                        pexp = pexp_pool.tile([128, ib], BF16, name="pexp")
                        nc.scalar.activation(pexp, psS, AF.Exp, scale=scale)
                        pexp_tiles.append(pexp)
                        # interleave PV chunks of the previous head so the
                        # single-buf po/trp psum pools have time to drain
                        pump_pv((jt + 1) * itpb // JT)
                        pump_a(idx + ahead)
                        if a_idle(idx):
                            pump_proj(proj_every)
                    # drain any PV leftovers of the previous head
                    pump_pv(10 ** 9)
                    last = (h == hpc - 1 and ibx == n_ibx - 1)
                    pv_state = {"gen": pv_gen(h, ibx, pexp_tiles), "done": 0,
                                "block_done":
                                    ibx if (h == hpc - 1 and not last)
                                    else None}
            # tail: interleave the last head's PV chunks with that block's
            # projection groups (4-deep psum rotation) so the drains pipeline
            last_bx = n_ibx - 1
            for ci in range(itpb):
                pump_pv(ci + 1)
                if ci >= 1:
                    # chunk ci-1's transpose+copy were emitted during step ci
                    for cc in range(dim // fc):
                        emit_proj_group(last_bx * itpb + ci - 1, cc,
                                        final=True)
                pump_proj(4)
            pump_pv(10 ** 9)
            for cc in range(dim // fc):
                emit_proj_group(last_bx * itpb + itpb - 1, cc, final=True)
            pump_proj(10 ** 9)
            pump_a(10 ** 9)


_BUILD_CACHE = {}


def build_nc(n=N_FULL, dim=DIM_FULL, hpc=HPC, ib=1024, ahead=0,
             proj_every=2):
    key = (n, dim, hpc, ib, ahead, proj_every)
    if key in _BUILD_CACHE:
        return _BUILD_CACHE[key]
    inner = hpc * DH
    KC = dim // 128
    S = inner // 128
    nc = bacc.Bacc("TRN2", target_bir_lowering=False, debug=False)
    xt = nc.dram_tensor("xt", [KC, 128, n], BF16, kind="ExternalInput").ap()
    wq = nc.dram_tensor("wq", [KC, 128, inner], BF16,
                        kind="ExternalInput").ap()
    wk = nc.dram_tensor("wk", [KC, 128, inner], BF16,
                        kind="ExternalInput").ap()
    wv = nc.dram_tensor("wv", [KC, 128, inner], BF16,
                        kind="ExternalInput").ap()
    wout = nc.dram_tensor("wout", [S, 128, dim], BF16,
                          kind="ExternalInput").ap()
    y = nc.dram_tensor("y", [n, dim], F32, kind="ExternalOutput").ap()
    with tile.TileContext(nc) as tc:
        with nc.allow_low_precision(
                reason="bf16 attention operands; fp32 accumulation"):
            emit_core_kernel(nc, tc, xt, wq, wk, wv, wout, y, n=n, dim=dim,
                             hpc=hpc, ib=ib, ahead=ahead,
                             proj_every=proj_every)
    nc.compile()
    _BUILD_CACHE[key] = nc
    return nc


def prep_core_inputs(x, w_qkv, w_out, n, dim, hpc):
    """Host-side prep for ONE core (layout only: transpose/chunk/cast)."""
    inner = hpc * DH
    KC = dim // 128
    x = np.asarray(x, np.float32)
    w_qkv = np.asarray(w_qkv, np.float32)
    bf = ml_dtypes.bfloat16
    xt = np.ascontiguousarray(x.T).reshape(KC, 128, n).astype(bf)
    return {
        "xt": xt,
        "wq": np.ascontiguousarray(
            w_qkv[:, 0:inner].reshape(KC, 128, inner)).astype(bf),
        "wk": np.ascontiguousarray(
            w_qkv[:, inner:2 * inner].reshape(KC, 128, inner)).astype(bf),
        "wv": np.ascontiguousarray(
            w_qkv[:, 2 * inner:3 * inner].reshape(KC, 128, inner)).astype(bf),
        "wout": np.ascontiguousarray(
            np.asarray(w_out, np.float32).reshape(
                inner // 128, 128, dim)).astype(bf),
    }


def make_in_maps(x, w_qkv, w_out):
    """Shard full inputs into the 8 per-core input maps."""
    x = np.asarray(x, dtype=np.float32)
    w_qkv = np.asarray(w_qkv, dtype=np.float32)
    w_out = np.asarray(w_out, dtype=np.float32)
    qk = HEADS_FULL * DH
    in_maps = []
    for core in range(N_CORES):
        b, g = divmod(core, GROUPS)
        cols = ts(g, INNER_PC)
        wqkv_c = np.concatenate(
            [w_qkv[:, cols],
             w_qkv[:, qk + g * INNER_PC:qk + (g + 1) * INNER_PC],
             w_qkv[:, 2 * qk + g * INNER_PC:2 * qk + (g + 1) * INNER_PC]],
            axis=1)
        in_maps.append(prep_core_inputs(x[b], wqkv_c, w_out[cols, :],
                                        N_FULL, DIM_FULL, HPC))
    return in_maps


def kernel(x, w_qkv, w_out, b_out, trace=False):
    b_out = np.asarray(b_out, dtype=np.float32)
    in_maps = make_in_maps(x, w_qkv, w_out)
    nc = build_nc()
    res = bass_utils.run_bass_kernel_spmd(
        nc, in_maps, core_ids=list(range(N_CORES)), trace=trace)
    ys = [r["y"] for r in res.results]
    out = np.empty((B_FULL, N_FULL, DIM_FULL), dtype=np.float32)
    for b in range(B_FULL):
        out[b] = ys[GROUPS * b] + ys[GROUPS * b + 1] + b_out[None, :]
    if trace:
        kernel.last_result = res
    return out


# revision 28
# speedup vs baseline: 1.0664x; 1.0167x over previous
"""Multi-head attention (b=4, n=2048, dim=1024, 16 heads x 64) on 8 Trainium2
NeuronCores.

Sharding: data-parallel over batch (4) x tensor-parallel over head-groups (2).
Each core gets one batch element and 8 heads; the host sums the two
head-group partials per batch element and adds b_out.

Per-core pipeline (bf16 operands everywhere; fp32 accumulation):
  A:  x^T arrives pre-transposed + pre-cast to bf16 from the host; [128, 512]
      slices are DMA'd straight into SBUF (no PE transposes).  qT / kT strips
      ([inner, n], bf16) and v (natural [n, inner] + a ones column per head,
      bf16) stay SBUF-resident.  Phase A's projection strips are WOVEN into
      phase B's step stream (each strip emitted just before its first
      consuming attention step), so the PE fills the startup DMA window with
      projection work instead of idling.
  B:  per (i-block, head): S^T j-tiles = matmul(lhsT=k^T_h j-block,
      rhs=q^T_h i-block); exp on ScalarE (1/sqrt(dh) folded into the
      activation scale) writes bf16 pexp tiles which stay resident for the
      whole head.  PV is FLIPPED: po[i-128, dh+1] psum accumulates
      matmul(lhsT=pexp_jt[:, i-chunk], rhs=v_jt[:, head cols]) over all 16
      j-tiles -- out free size is only dh+1=65, so PV costs ~2x fewer PE
      cycles than the [dh+1, i]-oriented version (cost model charges
      out-free-size per pass).  The last v column is ones, so po[:, dh] is
      the softmax denominator: DVE reciprocal + per-partition tensor_scalar
      multiply normalizes straight into a bf16 staging tile, PE transposes
      it (identity trick) to [dh, 128] psum, DVE copies into the oT strips.
      PV/normalize/transpose of head h-1 is interleaved into head h's score
      stream so each small psum pool (1 buf each) has time to drain.
  C:  y = O @ w_out via lhsT = oT strips (bf16), micro-woven (matmul at a
      time) into attention steps where the phase-A stream has nothing due;
      DVE drains the proj psums, y DMA'd out per 512-column chunk.
"""

from contextlib import ExitStack

import numpy as np
import ml_dtypes

import concourse.mybir as mybir
import concourse.tile as tile
from concourse import bacc, bass_utils, masks

F32 = mybir.dt.float32
BF16 = mybir.dt.bfloat16
F8 = mybir.dt.float8e4
DR = mybir.MatmulPerfMode.DoubleRow
AF = mybir.ActivationFunctionType
ALU = mybir.AluOpType

# (x-term, w-term) index pairs for the hi/lo fp8 product expansion
# x·w ~= xh·wh + xh·wl + xl·wh  (xl·wl ~ 0.1% of x·w, dropped)
HL_TERMS = ((0, 0), (0, 1), (1, 0))

# Pre-quantization scales so the lo residuals stay above e4m3's subnormal
# floor (2^-9): x ~ N(0,1) -> x*XS; w ~ N(0, 1/dim) -> w*WS.  Compensated in
# the exp activation scale (scores carry (XS*WS)^2) and in w_out (o carries
# XS*WS).
XS, WS = 4.0, 64.0

# Full-problem constants (hardcoded per the harness contract).
B_FULL, N_FULL, DIM_FULL = 4, 2048, 1024
HEADS_FULL, DH = 16, 64
N_CORES = 8
GROUPS = 2                       # head-group (tensor-parallel) factor
HPC = HEADS_FULL // GROUPS       # heads per core = 8
INNER_PC = HPC * DH              # per-core inner dim = 512


def ts(i, size):
    return slice(i * size, (i + 1) * size)


def emit_core_kernel(nc, tc, xt, wq, wk, wv, wout, y, *, n, dim, hpc,
                     ib=1024, ahead=0, proj_every=2):
    dh = DH
    inner = hpc * dh
    KC2 = dim // 256         # DoubleRow contraction chunk-pairs
    S = inner // 128         # 128-row strips of the inner dim
    JT = n // 128            # key/value j-tiles
    NB = n // 512            # 512-wide n-chunks in phase A
    ib = min(ib, n)
    n_ibx = n // ib
    itpb = ib // 128         # i-128 chunks per i-block
    cpb = ib // 512          # 512-chunks per i-block
    scale = float(1.0 / np.sqrt(dh) / (XS * WS) ** 2)
    fc = min(512, dim)
    assert n % 512 == 0 and dim % 128 == 0 and ib % 512 == 0

    stack = ExitStack()
    with stack:
        persist = stack.enter_context(tc.tile_pool(name="persist", bufs=1))
        w_pool = stack.enter_context(tc.tile_pool(name="a_w", bufs=1))
        xts_pool = stack.enter_context(
            tc.tile_pool(name="a_xts", bufs=KC2 * NB + 2))
        psA_pool = stack.enter_context(
            tc.tile_pool(name="a_ps", bufs=2, space="PSUM"))

        qT = [persist.tile([128, n], BF16, name="qT", tag=f"qT{s}")
              for s in range(S)]
        kT = [persist.tile([128, n], BF16, name="kT", tag=f"kT{s}")
              for s in range(S)]
        v_sb = [persist.tile([128, hpc * (dh + 1)], BF16, name="vt",
                             tag=f"v{j}") for j in range(JT)]
        for j in range(JT):
            nc.gpsimd.memset(
                v_sb[j].rearrange("p (h c) -> p h c", c=dh + 1)
                [:, :, dh:dh + 1], 1.0)
        oT = [persist.tile([128, n], BF16, name="oT", tag=f"oT{s}")
              for s in range(S)]
        ident = persist.tile([128, 128], BF16, name="ident")
        masks.make_identity(nc, ident)

        # weight DMAs: wk first (first strips), wv/wq next, wout last (only
        # needed by the first proj group, far in).
        # All projection operands are hi/lo fp8 pairs in DoubleRow layout:
        # x8 [KC2, 128, term, slot, n], w8 [KC2, 128, term, slot, inner].
        wq_sb, wk_sb, wv_sb, wout_sb = [], [], [], []
        xts_tiles = {}

        def xts(c, nb, eng=None):
            t = xts_tiles.get((c, nb))
            if t is None:
                t = xts_pool.tile([128, 2, 2, 512], F8, name="xts")
                if eng is None:
                    eng = nc.gpsimd if (c % 2) == 1 else nc.sync
                eng.dma_start(t, xt[c][:, :, :, ts(nb, 512)])
                xts_tiles[(c, nb)] = t
            return t

        # DMA priority order: strip-0 columns of wk (+x chunk 0) feed the
        # very first projection strips; then strip-0 of wq (+x chunk 1),
        # then wv (first v tile ~10 steps in), then the remaining w columns,
        # remaining x chunks, and wout (first consumed much later) last.
        for nm, lst in (("wk", wk_sb), ("wq", wq_sb)):
            if inner > 128:
                for c in range(KC2):
                    lst.append(w_pool.tile([128, 2, 2, inner - 128], F8,
                                           name="wt", tag=f"{nm}{c}"))
        for c in range(KC2):
            wv_sb.append(w_pool.tile([128, 2, 2, inner], F8, name="wt",
                                     tag=f"wv{c}"))
        # strip-0 weight columns for all chunk-pairs land as ONE strided DMA
        # each (the per-DMA queue cadence, not transfer size, paces startup)
        wks0 = w_pool.tile([128, KC2, 2, 2, 128], F8, name="wks0")
        for c in range(KC2):
            nc.sync.dma_start(wks0[:, c], wk[c][:, :, :, 0:128])
        for c in range(KC2):
            xts(c, 0)
        wqs0 = w_pool.tile([128, KC2, 2, 2, 128], F8, name="wqs0")
        for c in range(KC2):
            nc.sync.dma_start(wqs0[:, c], wq[c][:, :, :, 0:128])
        if NB > 1:
            for c in range(KC2):
                xts(c, 1)
        for c in range(KC2):
            nc.sync.dma_start(wv_sb[c], wv[c])
        for c in range(KC2):
            if inner > 128:
                nc.sync.dma_start(wk_sb[c], wk[c][:, :, :, 128:inner])
                nc.sync.dma_start(wq_sb[c], wq[c][:, :, :, 128:inner])
        for nb in range(2, NB):
            for c in range(KC2):
                xts(c, nb)
        for t in range(S):
            wo = persist.tile([128, dim], BF16, name="wo", tag=f"wo{t}")
            nc.sync.dma_start(wo, wout[t])
            wout_sb.append(wo)

        # ---- phase A ops (generators, woven into phase B at matmul
        #      granularity) ----
        def qk_strip_gen(w_sb, dst, t, nb):
            s0 = wks0 if w_sb is wk_sb else wqs0
            ps = psA_pool.tile([128, 512], F32, name="psA")
            nmm = KC2 * len(HL_TERMS)
            i = 0
            for c in range(KC2):
                for ta, tb in HL_TERMS:
                    lhsT = (s0[:, c, tb] if t == 0
                            else w_sb[c][:, tb, :, ts(t - 1, 128)])
                    nc.tensor.matmul(ps, lhsT, xts(c, nb)[:, ta],
                                     start=(i == 0), stop=(i == nmm - 1),
                                     perf_mode=DR)
                    i += 1
                    yield
            nc.vector.tensor_copy(dst[t][:, ts(nb, 512)], ps)

        def v_tile_gen(it):
            ps = psA_pool.tile([128, inner], F32, name="psA")
            nmm = KC2 * len(HL_TERMS)
            i = 0
            for c in range(KC2):
                for ta, tb in HL_TERMS:
                    nc.tensor.matmul(
                        ps, xts(c, it // 4)[:, ta, :, ts(it % 4, 128)],
                        wv_sb[c][:, tb],
                        start=(i == 0), stop=(i == nmm - 1), perf_mode=DR)
                    i += 1
                    yield
            nc.vector.tensor_copy(
                v_sb[it].rearrange("p (h c) -> p h c", c=dh + 1)[:, :, 0:dh],
                ps.rearrange("p (h d) -> p h d", d=dh))

        # need-index: flat B step index (ibx*hpc*JT + h*JT + jt) of the first
        # consumer of each A op.
        a_ops = []
        for s in range(S):
            for nb in range(NB):
                a_ops.append((2 * s * JT + 4 * nb, 0,
                              lambda s=s, nb=nb: qk_strip_gen(
                                  wk_sb, kT, s, nb)))
        for s in range(S):
            for ibx in range(n_ibx):
                for nb in range(ibx * ib // 512, (ibx + 1) * ib // 512):
                    a_ops.append((ibx * hpc * JT + 2 * s * JT, 1,
                                  lambda s=s, nb=nb: qk_strip_gen(
                                      wq_sb, qT, s, nb)))
        for it in range(JT):
            # v tiles are first read by PV of head 0, which is interleaved
            # into the SECOND half of head 1's score stream -- spread their
            # emission across heads 0 and 1 instead of bursting at head 0
            a_ops.append((1 + it * 3 // 2, 2, lambda it=it: v_tile_gen(it)))
        a_ops.sort(key=lambda x: (x[0], x[1]))
        a_state = {"ptr": 0, "gen": None}

        def a_step():
            """Advance the A stream one micro-op; False when exhausted."""
            while True:
                if a_state["gen"] is None:
                    if a_state["ptr"] >= len(a_ops):
                        return False
                    a_state["gen"] = a_ops[a_state["ptr"]][2]()
                try:
                    next(a_state["gen"])
                    return True
                except StopIteration:
                    a_state["gen"] = None
                    a_state["ptr"] += 1

        def pump_a(limit, budget=None):
            n_done = 0
            while True:
                if budget is not None and n_done >= budget:
                    return
                if a_state["ptr"] >= len(a_ops):
                    return
                if budget is None and a_ops[a_state["ptr"]][0] > limit:
                    return
                if not a_step():
                    return
                n_done += 1

        # ---- phase B/C ----
        with (
            tc.tile_pool(name="b_psS", bufs=2, space="PSUM") as psS_pool,
            # po ([128, dh+1] f32) and trp ([dh, 128] bf16) alternate through
            # the same two bank-sized slots: po(ci+1) waits only on po(ci)'s
            # DVE readers, trp(ci+1) only on trp(ci)'s DVE copy
            tc.tile_pool(name="b_psO", bufs=2, space="PSUM") as psO_pool,
            tc.tile_pool(name="b_pexp", bufs=2 * JT) as pexp_pool,
            tc.tile_pool(name="b_ost", bufs=3) as ost_pool,
            tc.tile_pool(name="b_rec", bufs=3) as rec_pool,
            tc.tile_pool(name="c_y", bufs=2) as y_pool,
        ):
            ysb_open = {}

            def emit_proj_group(it, cc, final=False):
                if cc == 0:
                    ysb_open[it] = y_pool.tile([128, dim], F32, name="ysb")
                ysb = ysb_open[it]
                if final:
                    # alternate psA with the (by now idle) psS slots so the
                    # last i-block's groups pipeline 4 deep
                    if (it * (dim // fc) + cc) % 2 == 0:
                        ps = psS_pool.tile([128, fc], F32, name="psS")
                    else:
                        ps = psA_pool.tile([128, fc], F32, name="psA")
                else:
                    ps = psA_pool.tile([128, fc], F32, name="psA")
                for t in range(S):
                    nc.tensor.matmul(
                        ps, oT[t][:, ts(it, 128)], wout_sb[t][:, ts(cc, fc)],
                        start=(t == 0), stop=(t == S - 1))
                nc.vector.tensor_copy(ysb[:, ts(cc, fc)], ps)
                nc.sync.dma_start(y[ts(it, 128), ts(cc, fc)],
                                  ysb[:, ts(cc, fc)])
                if cc == dim // fc - 1:
                    del ysb_open[it]

            proj_due = []
            proj_state = {"gen": None}

            def proj_group_gen(it, cc):
                if cc == 0:
                    ysb_open[it] = y_pool.tile([128, dim], F32, name="ysb")
                ysb = ysb_open[it]
                ps = psA_pool.tile([128, fc], F32, name="psA")
                for t in range(S):
                    nc.tensor.matmul(
                        ps, oT[t][:, ts(it, 128)], wout_sb[t][:, ts(cc, fc)],
                        start=(t == 0), stop=(t == S - 1))
                    yield
                nc.vector.tensor_copy(ysb[:, ts(cc, fc)], ps)
                nc.sync.dma_start(y[ts(it, 128), ts(cc, fc)],
                                  ysb[:, ts(cc, fc)])
                if cc == dim // fc - 1:
                    del ysb_open[it]

            def pump_proj(budget):
                n_done = 0
                while n_done < budget and (proj_state["gen"] or proj_due):
                    if proj_state["gen"] is None:
                        proj_state["gen"] = proj_group_gen(*proj_due.pop(0))
                    try:
                        next(proj_state["gen"])
                        n_done += 1
                    except StopIteration:
                        proj_state["gen"] = None

            def a_idle(idx):
                return (a_state["ptr"] >= len(a_ops)
                        and a_state["gen"] is None) or (
                    a_state["gen"] is None
                    and a_ops[a_state["ptr"]][0] > idx + ahead)

            def pv_gen(h, ibx, pexp_tiles):
                """Flipped PV + normalize + transpose for one (head, i-block).
                One yield per i-128 chunk; each chunk's transpose+copy are
                deferred to the NEXT step so the PE never waits on the
                freshly-issued DVE recip/normalize chain."""
                s_, r_ = divmod(h * dh, 128)
                vcol = slice(h * (dh + 1), (h + 1) * (dh + 1))

                def finish(ost, ci):
                    c0 = ibx * ib + ci * 128
                    trp = psO_pool.tile([dh, 128], BF16, name="trp", tag="pot")
                    nc.tensor.transpose(trp, ost, ident)
                    nc.vector.tensor_copy(oT[s_][r_:r_ + dh, c0:c0 + 128],
                                          trp)

                prev = None
                for ci in range(itpb):
                    if prev is not None:
                        finish(*prev)
                    po = psO_pool.tile([128, dh + 1], F32, name="po", tag="pot")
                    for jt in range(JT):
                        nc.tensor.matmul(
                            po, pexp_tiles[jt][:, ts(ci, 128)],
                            v_sb[jt][:, vcol],
                            start=(jt == 0), stop=(jt == JT - 1))
                    recip = rec_pool.tile([128, 1], F32, name="recip")
                    nc.vector.reciprocal(recip, po[:, dh:dh + 1])
                    ost = ost_pool.tile([128, dh], BF16, name="ost")
                    nc.vector.tensor_scalar_mul(ost, po[:, 0:dh], recip)
                    prev = (ost, ci)
                    yield
                finish(*prev)

            pv_state = {"gen": None, "done": 0, "block_done": None}
            _DONE = object()

            def pump_pv(want):
                st = pv_state
                while st["gen"] is not None and st["done"] < want:
                    if next(st["gen"], _DONE) is _DONE:
                        st["gen"] = None
                        if st["block_done"] is not None:
                            # last head of block finished: its out-projection
                            # groups may now be emitted (all oT writes for the
                            # block precede them in program order)
                            bx = st["block_done"]
                            proj_due.extend(
                                (it, cc)
                                for it in range(bx * itpb, (bx + 1) * itpb)
                                for cc in range(dim // fc))
                        break
                    st["done"] += 1

            for ibx in range(n_ibx):
                for h in range(hpc):
                    s_, r_ = divmod(h * dh, 128)
                    kTh = kT[s_][r_:r_ + dh, :]
                    qTh = qT[s_][r_:r_ + dh, :]
                    pexp_tiles = []
                    for jt in range(JT):
                        idx = ibx * hpc * JT + h * JT + jt
                        pump_a(idx)
                        psS = psS_pool.tile([128, ib], F32, name="psS")
                        for cc in range(cpb):
                            nc.tensor.matmul(
                                psS[:, ts(cc, 512)], kTh[:, ts(jt, 128)],
                                qTh[:, ibx * ib + cc * 512:
                                    ibx * ib + (cc + 1) * 512],
                                start=True, stop=True)
                        pexp = pexp_pool.tile([128, ib], BF16, name="pexp")
                        nc.scalar.activation(pexp, psS, AF.Exp, scale=scale)
                        pexp_tiles.append(pexp)
                        # interleave PV chunks of the previous head into the
                        # SECOND half of this head's score stream (the v tiles
                        # PV reads are only projected by then, and the po/trp
                        # psum slots get time to drain between chunks)
                        h2 = JT - JT // 2
                        pump_pv(max(0, (jt + 1 - JT // 2) * itpb // h2))
                        pump_a(idx + ahead)
                        if a_idle(idx):
                            pump_proj(proj_every)
                    # drain any PV leftovers of the previous head
                    pump_pv(10 ** 9)
                    last = (h == hpc - 1 and ibx == n_ibx - 1)
                    pv_state = {"gen": pv_gen(h, ibx, pexp_tiles), "done": 0,
                                "block_done":
                                    ibx if (h == hpc - 1 and not last)
                                    else None}
            # tail: interleave the last head's PV chunks with that block's
            # projection groups (4-deep psum rotation) so the drains pipeline
            last_bx = n_ibx - 1
            for ci in range(itpb):
                pump_pv(ci + 1)
                if ci >= 1:
                    # chunk ci-1's transpose+copy were emitted during step ci
                    for cc in range(dim // fc):
                        emit_proj_group(last_bx * itpb + ci - 1, cc,
                                        final=True)
                pump_proj(4)
            pump_pv(10 ** 9)
            for cc in range(dim // fc):
                emit_proj_group(last_bx * itpb + itpb - 1, cc, final=True)
            pump_proj(10 ** 9)
            pump_a(10 ** 9)


_BUILD_CACHE = {}


def build_nc(n=N_FULL, dim=DIM_FULL, hpc=HPC, ib=1024, ahead=0,
             proj_every=2):
    key = (n, dim, hpc, ib, ahead, proj_every)
    if key in _BUILD_CACHE:
        return _BUILD_CACHE[key]
    inner = hpc * DH
    KC2 = dim // 256
    S = inner // 128
    nc = bacc.Bacc("TRN2", target_bir_lowering=False, debug=False)
    xt = nc.dram_tensor("xt", [KC2, 128, 2, 2, n], F8,
                        kind="ExternalInput").ap()
    wq = nc.dram_tensor("wq", [KC2, 128, 2, 2, inner], F8,
                        kind="ExternalInput").ap()
    wk = nc.dram_tensor("wk", [KC2, 128, 2, 2, inner], F8,
                        kind="ExternalInput").ap()
    wv = nc.dram_tensor("wv", [KC2, 128, 2, 2, inner], F8,
                        kind="ExternalInput").ap()
    wout = nc.dram_tensor("wout", [S, 128, dim], BF16,
                          kind="ExternalInput").ap()
    y = nc.dram_tensor("y", [n, dim], F32, kind="ExternalOutput").ap()
    with tile.TileContext(nc) as tc:
        with nc.allow_low_precision(
                reason="bf16 attention operands; fp32 accumulation"):
            emit_core_kernel(nc, tc, xt, wq, wk, wv, wout, y, n=n, dim=dim,
                             hpc=hpc, ib=ib, ahead=ahead,
                             proj_every=proj_every)
    nc.compile()
    _BUILD_CACHE[key] = nc
    return nc


def _hilo_dr(a, dim, ncols):
    """[dim, ncols] f32 -> [dim/256, 128, 2(term hi/lo), 2(slot), ncols] fp8.

    term 0/1 = hi/lo of the value; slot i pairs dim rows (c*256 + i*128 + p)
    for the DoubleRow 256-deep contraction."""
    f8 = ml_dtypes.float8_e4m3
    hi = a.astype(f8)
    lo = (a - hi.astype(np.float32)).astype(f8)
    KC2 = dim // 256
    # [dim, ncols] -> [KC2, 2(slot), 128, ncols] -> [KC2, 128, 2slot, ncols]
    def arr(t):
        return t.reshape(KC2, 2, 128, ncols).transpose(0, 2, 1, 3)
    out = np.stack([arr(hi), arr(lo)], axis=2)  # [KC2, 128, term, slot, cols]
    return np.ascontiguousarray(out)


def prep_core_inputs(x, w_qkv, w_out, n, dim, hpc):
    """Host-side prep for ONE core (layout only: transpose/chunk/cast)."""
    inner = hpc * DH
    x = np.asarray(x, np.float32)
    w_qkv = np.asarray(w_qkv, np.float32)
    bf = ml_dtypes.bfloat16
    xT = np.ascontiguousarray(x.T) * np.float32(XS)
    return {
        "xt": _hilo_dr(xT, dim, n),
        "wq": _hilo_dr(w_qkv[:, 0:inner] * np.float32(WS), dim, inner),
        "wk": _hilo_dr(w_qkv[:, inner:2 * inner] * np.float32(WS), dim,
                       inner),
        "wv": _hilo_dr(w_qkv[:, 2 * inner:3 * inner] * np.float32(WS), dim,
                       inner),
        "wout": np.ascontiguousarray(
            np.asarray(w_out, np.float32).reshape(
                inner // 128, 128, dim) / np.float32(XS * WS)).astype(bf),
    }


def make_in_maps(x, w_qkv, w_out):
    """Shard full inputs into the 8 per-core input maps."""
    x = np.asarray(x, dtype=np.float32)
    w_qkv = np.asarray(w_qkv, dtype=np.float32)
    w_out = np.asarray(w_out, dtype=np.float32)
    qk = HEADS_FULL * DH
    in_maps = []
    for core in range(N_CORES):
        b, g = divmod(core, GROUPS)
        cols = ts(g, INNER_PC)
        wqkv_c = np.concatenate(
            [w_qkv[:, cols],
             w_qkv[:, qk + g * INNER_PC:qk + (g + 1) * INNER_PC],
             w_qkv[:, 2 * qk + g * INNER_PC:2 * qk + (g + 1) * INNER_PC]],
            axis=1)
        in_maps.append(prep_core_inputs(x[b], wqkv_c, w_out[cols, :],
                                        N_FULL, DIM_FULL, HPC))
    return in_maps


def kernel(x, w_qkv, w_out, b_out, trace=False):
    b_out = np.asarray(b_out, dtype=np.float32)
    in_maps = make_in_maps(x, w_qkv, w_out)
    nc = build_nc()
    res = bass_utils.run_bass_kernel_spmd(
        nc, in_maps, core_ids=list(range(N_CORES)), trace=trace)
    ys = [r["y"] for r in res.results]
    out = np.empty((B_FULL, N_FULL, DIM_FULL), dtype=np.float32)
    for b in range(B_FULL):
        out[b] = ys[GROUPS * b] + ys[GROUPS * b + 1] + b_out[None, :]
    if trace:
        kernel.last_result = res
    return out


# revision 48
# speedup vs baseline: 1.0703x; 1.0036x over previous
"""Multi-head attention (b=4, n=2048, dim=1024, 16 heads x 64) on 8 Trainium2
NeuronCores.

Sharding: data-parallel over batch (4) x tensor-parallel over head-groups (2).
Each core gets one batch element and 8 heads; the host sums the two
head-group partials per batch element and adds b_out.

Per-core pipeline (fp8 hi/lo DoubleRow projections, bf16 attention, fp32
accumulation):
  A:  x^T and the qkv weights arrive from the host as hi/lo fp8e4 pairs in
      DoubleRow slot layout (x*4, w*64 pre-scales keep the lo residuals
      above e4m3's subnormal floor; compensated in the exp scale and w_out).
      Each projection accumulates 3 cross terms (xh*wh + xh*wl + xl*wh) per
      256-deep DoubleRow chunk-pair at 0.5 cycles/row -- 25% fewer PE cycles
      than bf16, and ~2x less quantization error.  qT / kT strips
      ([inner, n], bf16) and v (natural [n, inner] + a ones column per head,
      bf16) stay SBUF-resident.  Phase A's projection strips are WOVEN into
      phase B's step stream (each strip emitted just before its first
      consuming attention step), so the PE fills the startup DMA window with
      projection work instead of idling.
  B:  per (i-block, head): S^T j-tiles = matmul(lhsT=k^T_h j-block,
      rhs=q^T_h i-block); exp on ScalarE (1/sqrt(dh) folded into the
      activation scale) writes bf16 pexp tiles which stay resident for the
      whole head.  PV is FLIPPED: po[i-128, dh+1] psum accumulates
      matmul(lhsT=pexp_jt[:, i-chunk], rhs=v_jt[:, head cols]) over all 16
      j-tiles -- out free size is only dh+1=65, so PV costs ~2x fewer PE
      cycles than the [dh+1, i]-oriented version (cost model charges
      out-free-size per pass).  The last v column is ones, so po[:, dh] is
      the softmax denominator: DVE reciprocal + per-partition tensor_scalar
      multiply normalizes straight into a bf16 staging tile, PE transposes
      it (identity trick) to [dh, 128] psum, DVE copies into the oT strips.
      PV/normalize/transpose of head h-1 is interleaved into head h's score
      stream so each small psum pool (1 buf each) has time to drain.
  C:  y = O @ w_out via lhsT = oT strips (bf16), micro-woven (matmul at a
      time) into attention steps where the phase-A stream has nothing due;
      DVE drains the proj psums, y DMA'd out per 512-column chunk.
"""

from contextlib import ExitStack

import numpy as np
import ml_dtypes

import concourse.mybir as mybir
import concourse.tile as tile
from concourse import bacc, bass_utils, masks
from concourse import dve_ops as _dvo
from concourse.dve_spec import (
    C0 as _C0, C1 as _C1, C2 as _C2, One as _One,
    AluOp as _AluOp, Bin as _Bin, Spec as _Spec, Src0 as _Src0,
)


def _sq(x):
    return _Bin(_AluOp.MULTIPLY, x, x)


def _exp2a_ref(in0, in1, s0, s1, imm2):
    u = in0.astype(np.float32) * np.float32(s0)
    p = (np.float32(s1) * u + np.float32(imm2)) * u + np.float32(1.0)
    p = (p * p).astype(np.float32)
    return (p * p).astype(np.float32)


def _exp2b_ref(in0, in1, s0, s1, imm2):
    x = in0.astype(np.float32)
    for _ in range(4):
        x = (x * x).astype(np.float32)
    return x


# exp(logit) = 2^(64u) = (quad(u))^64 with u = logit*log2e/64: op A scales,
# evaluates the minimax quadratic for 2^u (|u| <= ~0.2) and squares twice;
# op B squares four more times.  Max rel err ~0.55% for |logit| <= 8 -- used
# on a fraction of the softmax tiles to offload ScalarE.
_u_ = _Src0 * _C0
_p_ = (_C1 * _u_ + _C2) * _u_ + _One
EXP2A = _dvo.DveOp("EXP2_LADDER_A", _Spec(body=_sq(_sq(_p_)),
                                          reference=_exp2a_ref),
                   subdim=False, uops_sha={"v3": "c3940ebb62d9d92f"})
EXP2B = _dvo.DveOp("EXP2_LADDER_B", _Spec(body=_sq(_sq(_sq(_sq(_Src0)))),
                                          reference=_exp2b_ref),
                   subdim=False, uops_sha={"v3": "6d6edb7498c4a68d"})
if "EXP2_LADDER_A" not in _dvo._SUB_OPCODE_FOR_NAME:
    _dvo._SUB_OPCODE_FOR_NAME["EXP2_LADDER_A"] = 17
    _dvo._SUB_OPCODE_FOR_NAME["EXP2_LADDER_B"] = 18
    _dvo.OPS.append(EXP2A)
    _dvo.OPS.append(EXP2B)
    _dvo.CUSTOM_DVE_SPECS["EXP2_LADDER_A"] = EXP2A.spec
    _dvo.CUSTOM_DVE_SPECS["EXP2_LADDER_B"] = EXP2B.spec

# minimax quadratic coefficients for 2^u, |u| <= 8*log2e/64
EXP2_C1 = 0.6944773368902336
EXP2_C2 = 0.2402110103092763

# offload every k-th softmax tile's exp to the DVE ladder (0 = all ScalarE)
import os as _os
DVE_EVERY_DEFAULT = int(_os.environ.get("DVE_EVERY", "0"))

F32 = mybir.dt.float32
BF16 = mybir.dt.bfloat16
F8 = mybir.dt.float8e4
DR = mybir.MatmulPerfMode.DoubleRow
AF = mybir.ActivationFunctionType
ALU = mybir.AluOpType

# (x-term, w-term) index pairs for the hi/lo fp8 product expansion
# x·w ~= xh·wh + xh·wl + xl·wh  (xl·wl ~ 0.1% of x·w, dropped)
HL_TERMS = ((0, 0), (0, 1), (1, 0))

# Pre-quantization scales so the lo residuals stay above e4m3's subnormal
# floor (2^-9): x ~ N(0,1) -> x*XS; w ~ N(0, 1/dim) -> w*WS.  Compensated in
# the exp activation scale (scores carry (XS*WS)^2) and in w_out (o carries
# XS*WS).
XS, WS = 4.0, 64.0

# Full-problem constants (hardcoded per the harness contract).
B_FULL, N_FULL, DIM_FULL = 4, 2048, 1024
HEADS_FULL, DH = 16, 64
N_CORES = 8
GROUPS = 2                       # head-group (tensor-parallel) factor
HPC = HEADS_FULL // GROUPS       # heads per core = 8
INNER_PC = HPC * DH              # per-core inner dim = 512


def ts(i, size):
    return slice(i * size, (i + 1) * size)


def emit_core_kernel(nc, tc, xt, wq, wk, wv, wout, y, *, n, dim, hpc,
                     ib=1024, ahead=0, proj_every=2, dve_every=0):
    dh = DH
    inner = hpc * dh
    KC2 = dim // 256         # DoubleRow contraction chunk-pairs
    S = inner // 128         # 128-row strips of the inner dim
    JT = n // 128            # key/value j-tiles
    NB = n // 512            # 512-wide n-chunks in phase A
    ib = min(ib, n)
    n_ibx = n // ib
    itpb = ib // 128         # i-128 chunks per i-block
    cpb = ib // 512          # 512-chunks per i-block
    scale = float(1.0 / np.sqrt(dh) / (XS * WS) ** 2)
    fc = min(512, dim)
    assert n % 512 == 0 and dim % 128 == 0 and ib % 512 == 0

    stack = ExitStack()
    with stack:
        persist = stack.enter_context(tc.tile_pool(name="persist", bufs=1))
        w_pool = stack.enter_context(tc.tile_pool(name="a_w", bufs=1))
        xts_pool = stack.enter_context(
            tc.tile_pool(name="a_xts", bufs=KC2 * NB))
        psA_pool = stack.enter_context(
            tc.tile_pool(name="a_ps", bufs=2, space="PSUM"))

        qT = [persist.tile([128, n], BF16, name="qT", tag=f"qT{s}")
              for s in range(S)]
        kT = [persist.tile([128, n], BF16, name="kT", tag=f"kT{s}")
              for s in range(S)]
        v_sb = [persist.tile([128, hpc * (dh + 1)], BF16, name="vt",
                             tag=f"v{j}") for j in range(JT)]
        for j in range(JT):
            nc.gpsimd.memset(
                v_sb[j].rearrange("p (h c) -> p h c", c=dh + 1)
                [:, :, dh:dh + 1], 1.0)
        oT = [persist.tile([128, n], BF16, name="oT", tag=f"oT{s}")
              for s in range(S)]
        ident = persist.tile([128, 128], BF16, name="ident")
        masks.make_identity(nc, ident)

        # weight DMAs: wk first (first strips), wv/wq next, wout last (only
        # needed by the first proj group, far in).
        # All projection operands are hi/lo fp8 pairs in DoubleRow layout:
        # x8 [KC2, 128, term, slot, n], w8 [KC2, 128, term, slot, inner].
        wq_sb, wk_sb, wv_sb, wout_sb = [], [], [], []
        xts_tiles = {}

        def xts(c, nb, eng=None):
            t = xts_tiles.get((c, nb))
            if t is None:
                t = xts_pool.tile([128, 2, 2, 512], F8, name="xts")
                if eng is None:
                    eng = nc.gpsimd if (c % 2) == 1 else nc.sync
                eng.dma_start(t, xt[c][:, :, :, ts(nb, 512)])
                xts_tiles[(c, nb)] = t
            return t

        # DMA priority order: strip-0 columns of wk (+x chunk 0) feed the
        # very first projection strips; then strip-0 of wq (+x chunk 1),
        # then wv (first v tile ~10 steps in), then the remaining w columns,
        # remaining x chunks, and wout (first consumed much later) last.
        for nm, lst in (("wk", wk_sb), ("wq", wq_sb)):
            if inner > 128:
                for c in range(KC2):
                    lst.append(w_pool.tile([128, 2, 2, inner - 128], F8,
                                           name="wt", tag=f"{nm}{c}"))
        for c in range(KC2):
            wv_sb.append(w_pool.tile([128, 2, 2, inner], F8, name="wt",
                                     tag=f"wv{c}"))
        # strip-0 weight columns for all chunk-pairs land as ONE strided DMA
        # each (the per-DMA queue cadence, not transfer size, paces startup)
        wks0 = w_pool.tile([128, KC2, 2, 2, 128], F8, name="wks0")
        for c in range(KC2):
            nc.sync.dma_start(wks0[:, c], wk[c][:, :, :, 0:128])
            xts(c, 0)
        wqs0 = w_pool.tile([128, KC2, 2, 2, 128], F8, name="wqs0")
        for c in range(KC2):
            (nc.sync if c % 2 == 0 else nc.gpsimd).dma_start(
                wqs0[:, c], wq[c][:, :, :, 0:128])
            if NB > 1:
                xts(c, 1)
        for c in range(KC2):
            nc.sync.dma_start(wv_sb[c], wv[c])
        for c in range(KC2):
            if inner > 128:
                nc.sync.dma_start(wk_sb[c], wk[c][:, :, :, 128:inner])
                nc.sync.dma_start(wq_sb[c], wq[c][:, :, :, 128:inner])
        for nb in range(2, NB):
            for c in range(KC2):
                xts(c, nb)
        for t in range(S):
            wo = persist.tile([128, dim], BF16, name="wo", tag=f"wo{t}")
            nc.sync.dma_start(wo, wout[t])
            wout_sb.append(wo)

        # ---- phase A ops (generators, woven into phase B at matmul
        #      granularity) ----
        def qk_strip_gen(w_sb, dst, t, nb):
            s0 = wks0 if w_sb is wk_sb else wqs0
            ps = psA_pool.tile([128, 512], F32, name="psA")
            nmm = KC2 * len(HL_TERMS)
            i = 0
            for c in range(KC2):
                for ta, tb in HL_TERMS:
                    lhsT = (s0[:, c, tb] if t == 0
                            else w_sb[c][:, tb, :, ts(t - 1, 128)])
                    nc.tensor.matmul(ps, lhsT, xts(c, nb)[:, ta],
                                     start=(i == 0), stop=(i == nmm - 1),
                                     perf_mode=DR)
                    i += 1
                    yield
            nc.vector.tensor_copy(dst[t][:, ts(nb, 512)], ps)

        def v_tile_gen(it):
            ps = psA_pool.tile([128, inner], F32, name="psA")
            nmm = KC2 * len(HL_TERMS)
            i = 0
            for c in range(KC2):
                for ta, tb in HL_TERMS:
                    nc.tensor.matmul(
                        ps, xts(c, it // 4)[:, ta, :, ts(it % 4, 128)],
                        wv_sb[c][:, tb],
                        start=(i == 0), stop=(i == nmm - 1), perf_mode=DR)
                    i += 1
                    yield
            nc.vector.tensor_copy(
                v_sb[it].rearrange("p (h c) -> p h c", c=dh + 1)[:, :, 0:dh],
                ps.rearrange("p (h d) -> p h d", d=dh))

        # need-index: flat B step index (ibx*hpc*JT + h*JT + jt) of the first
        # consumer of each A op.
        a_ops = []
        for s in range(S):
            for nb in range(NB):
                a_ops.append((2 * s * JT + 4 * nb, 0,
                              lambda s=s, nb=nb: qk_strip_gen(
                                  wk_sb, kT, s, nb)))
        for s in range(S):
            for ibx in range(n_ibx):
                for nb in range(ibx * ib // 512, (ibx + 1) * ib // 512):
                    a_ops.append((ibx * hpc * JT + 2 * s * JT, 1,
                                  lambda s=s, nb=nb: qk_strip_gen(
                                      wq_sb, qT, s, nb)))
        for it in range(JT):
            # v tiles are first read by PV of head 0, which is interleaved
            # into the SECOND half of head 1's score stream -- spread their
            # emission across heads 0 and 1 instead of bursting at head 0
            a_ops.append((1 + it * 3 // 2, 2, lambda it=it: v_tile_gen(it)))
        a_ops.sort(key=lambda x: (x[0], x[1]))
        a_state = {"ptr": 0, "gen": None}

        def a_step():
            """Advance the A stream one micro-op; False when exhausted."""
            while True:
                if a_state["gen"] is None:
                    if a_state["ptr"] >= len(a_ops):
                        return False
                    a_state["gen"] = a_ops[a_state["ptr"]][2]()
                try:
                    next(a_state["gen"])
                    return True
                except StopIteration:
                    a_state["gen"] = None
                    a_state["ptr"] += 1

        def pump_a(limit, budget=None):
            n_done = 0
            while True:
                if budget is not None and n_done >= budget:
                    return
                if a_state["ptr"] >= len(a_ops):
                    return
                if budget is None and a_ops[a_state["ptr"]][0] > limit:
                    return
                if not a_step():
                    return
                n_done += 1

        # ---- phase B/C ----
        with (
            tc.tile_pool(name="b_psS", bufs=2, space="PSUM") as psS_pool,
            # po ([128, dh+1] f32) and trp ([dh, 128] bf16) alternate through
            # the same two bank-sized slots: po(ci+1) waits only on po(ci)'s
            # DVE readers, trp(ci+1) only on trp(ci)'s DVE copy
            tc.tile_pool(name="b_psO", bufs=2, space="PSUM") as psO_pool,
            tc.tile_pool(name="b_pexp", bufs=2 * JT + 2) as pexp_pool,
            tc.tile_pool(name="b_ost", bufs=3) as ost_pool,
            tc.tile_pool(name="b_rec", bufs=3) as rec_pool,
            tc.tile_pool(name="c_y", bufs=2) as y_pool,
        ):
            ysb_open = {}

            def emit_proj_group(it, cc, final=False):
                if cc == 0:
                    ysb_open[it] = y_pool.tile([128, dim], F32, name="ysb")
                ysb = ysb_open[it]
                if final:
                    # alternate psA with the (by now idle) psS slots so the
                    # last i-block's groups pipeline 4 deep
                    if (it * (dim // fc) + cc) % 2 == 0:
                        ps = psS_pool.tile([128, fc], F32, name="psS")
                    else:
                        ps = psA_pool.tile([128, fc], F32, name="psA")
                else:
                    ps = psA_pool.tile([128, fc], F32, name="psA")
                for t in range(S):
                    nc.tensor.matmul(
                        ps, oT[t][:, ts(it, 128)], wout_sb[t][:, ts(cc, fc)],
                        start=(t == 0), stop=(t == S - 1))
                nc.vector.tensor_copy(ysb[:, ts(cc, fc)], ps)
                nc.sync.dma_start(y[ts(it, 128), ts(cc, fc)],
                                  ysb[:, ts(cc, fc)])
                if cc == dim // fc - 1:
                    del ysb_open[it]

            proj_due = []
            proj_state = {"gen": None}

            def proj_group_gen(it, cc):
                if cc == 0:
                    ysb_open[it] = y_pool.tile([128, dim], F32, name="ysb")
                ysb = ysb_open[it]
                ps = psA_pool.tile([128, fc], F32, name="psA")
                for t in range(S):
                    nc.tensor.matmul(
                        ps, oT[t][:, ts(it, 128)], wout_sb[t][:, ts(cc, fc)],
                        start=(t == 0), stop=(t == S - 1))
                    yield
                nc.vector.tensor_copy(ysb[:, ts(cc, fc)], ps)
                nc.sync.dma_start(y[ts(it, 128), ts(cc, fc)],
                                  ysb[:, ts(cc, fc)])
                if cc == dim // fc - 1:
                    del ysb_open[it]

            def pump_proj(budget):
                n_done = 0
                while n_done < budget and (proj_state["gen"] or proj_due):
                    if proj_state["gen"] is None:
                        proj_state["gen"] = proj_group_gen(*proj_due.pop(0))
                    try:
                        next(proj_state["gen"])
                        n_done += 1
                    except StopIteration:
                        proj_state["gen"] = None

            def a_idle(idx):
                return (a_state["ptr"] >= len(a_ops)
                        and a_state["gen"] is None) or (
                    a_state["gen"] is None
                    and a_ops[a_state["ptr"]][0] > idx + ahead)

            def pv_gen(h, ibx, pexp_tiles):
                """Flipped PV + normalize + transpose for one (head, i-block).
                One yield per i-128 chunk; each chunk's transpose+copy are
                deferred to the NEXT step so the PE never waits on the
                freshly-issued DVE recip/normalize chain."""
                s_, r_ = divmod(h * dh, 128)
                vcol = slice(h * (dh + 1), (h + 1) * (dh + 1))

                def finish(ost, ci):
                    c0 = ibx * ib + ci * 128
                    trp = psO_pool.tile([dh, 128], BF16, name="trp", tag="pot")
                    nc.tensor.transpose(trp, ost, ident)
                    nc.vector.tensor_copy(oT[s_][r_:r_ + dh, c0:c0 + 128],
                                          trp)

                prev = None
                for ci in range(itpb):
                    if prev is not None:
                        finish(*prev)
                    po = psO_pool.tile([128, dh + 1], F32, name="po", tag="pot")
                    for jt in range(JT):
                        nc.tensor.matmul(
                            po, pexp_tiles[jt][:, ts(ci, 128)],
                            v_sb[jt][:, vcol],
                            start=(jt == 0), stop=(jt == JT - 1))
                    recip = rec_pool.tile([128, 1], F32, name="recip")
                    nc.vector.reciprocal(recip, po[:, dh:dh + 1])
                    ost = ost_pool.tile([128, dh], BF16, name="ost")
                    nc.vector.tensor_scalar_mul(ost, po[:, 0:dh], recip)
                    prev = (ost, ci)
                    yield
                finish(*prev)

            pv_state = {"gen": None, "done": 0, "block_done": None}
            _DONE = object()

            def pump_pv(want):
                st = pv_state
                while st["gen"] is not None and st["done"] < want:
                    if next(st["gen"], _DONE) is _DONE:
                        st["gen"] = None
                        if st["block_done"] is not None:
                            # last head of block finished: its out-projection
                            # groups may now be emitted (all oT writes for the
                            # block precede them in program order)
                            bx = st["block_done"]
                            proj_due.extend(
                                (it, cc)
                                for it in range(bx * itpb, (bx + 1) * itpb)
                                for cc in range(dim // fc))
                        break
                    st["done"] += 1

            for ibx in range(n_ibx):
                for h in range(hpc):
                    s_, r_ = divmod(h * dh, 128)
                    kTh = kT[s_][r_:r_ + dh, :]
                    qTh = qT[s_][r_:r_ + dh, :]
                    pexp_tiles = []
                    for jt in range(JT):
                        idx = ibx * hpc * JT + h * JT + jt
                        pump_a(idx)
                        psS = psS_pool.tile([128, ib], F32, name="psS")
                        for cc in range(cpb):
                            nc.tensor.matmul(
                                psS[:, ts(cc, 512)], kTh[:, ts(jt, 128)],
                                qTh[:, ibx * ib + cc * 512:
                                    ibx * ib + (cc + 1) * 512],
                                start=True, stop=True)
                        pexp = pexp_pool.tile([128, ib], BF16, name="pexp")
                        if dve_every and jt % dve_every == dve_every - 1:
                            nc.vector._custom_dve(
                                EXP2A, out=psS, in0=psS,
                                s0=float(scale * np.log2(np.e) / 64.0),
                                s1=EXP2_C1, imm2=EXP2_C2)
                            nc.vector._custom_dve(EXP2B, out=pexp, in0=psS)
                        else:
                            nc.scalar.activation(pexp, psS, AF.Exp,
                                                 scale=scale)
                        pexp_tiles.append(pexp)
                        # interleave PV chunks of the previous head into the
                        # SECOND half of this head's score stream (the v tiles
                        # PV reads are only projected by then, and the po/trp
                        # psum slots get time to drain between chunks)
                        st = 3 * JT // 8
                        pump_pv(max(0, (jt + 1 - st) * itpb // (JT - st)))
                        pump_a(idx + ahead)
                        if a_idle(idx):
                            pump_proj(proj_every)
                    # drain any PV leftovers of the previous head
                    pump_pv(10 ** 9)
                    last = (h == hpc - 1 and ibx == n_ibx - 1)
                    pv_state = {"gen": pv_gen(h, ibx, pexp_tiles), "done": 0,
                                "block_done":
                                    ibx if (h == hpc - 1 and not last)
                                    else None}
            # tail: interleave the last head's PV chunks with that block's
            # projection groups (4-deep psum rotation) so the drains pipeline
            last_bx = n_ibx - 1
            for ci in range(itpb):
                pump_pv(ci + 1)
                if ci >= 1:
                    # chunk ci-1's transpose+copy were emitted during step ci
                    for cc in range(dim // fc):
                        emit_proj_group(last_bx * itpb + ci - 1, cc,
                                        final=True)
                pump_proj(4)
            pump_pv(10 ** 9)
            for cc in range(dim // fc):
                emit_proj_group(last_bx * itpb + itpb - 1, cc, final=True)
            pump_proj(10 ** 9)
            pump_a(10 ** 9)


_BUILD_CACHE = {}


def build_nc(n=N_FULL, dim=DIM_FULL, hpc=HPC, ib=1024, ahead=0,
             proj_every=2, dve_every=DVE_EVERY_DEFAULT):
    key = (n, dim, hpc, ib, ahead, proj_every, dve_every)
    if key in _BUILD_CACHE:
        return _BUILD_CACHE[key]
    inner = hpc * DH
    KC2 = dim // 256
    S = inner // 128
    nc = bacc.Bacc("TRN2", target_bir_lowering=False, debug=False)
    xt = nc.dram_tensor("xt", [KC2, 128, 2, 2, n], F8,
                        kind="ExternalInput").ap()
    wq = nc.dram_tensor("wq", [KC2, 128, 2, 2, inner], F8,
                        kind="ExternalInput").ap()
    wk = nc.dram_tensor("wk", [KC2, 128, 2, 2, inner], F8,
                        kind="ExternalInput").ap()
    wv = nc.dram_tensor("wv", [KC2, 128, 2, 2, inner], F8,
                        kind="ExternalInput").ap()
    wout = nc.dram_tensor("wout", [S, 128, dim], BF16,
                          kind="ExternalInput").ap()
    y = nc.dram_tensor("y", [n, dim], F32, kind="ExternalOutput").ap()
    with tile.TileContext(nc) as tc:
        with nc.allow_low_precision(
                reason="bf16 attention operands; fp32 accumulation"):
            emit_core_kernel(nc, tc, xt, wq, wk, wv, wout, y, n=n, dim=dim,
                             hpc=hpc, ib=ib, ahead=ahead,
                             proj_every=proj_every, dve_every=dve_every)
    nc.compile()
    _BUILD_CACHE[key] = nc
    return nc


def _hilo_dr(a, dim, ncols):
    """[dim, ncols] f32 -> [dim/256, 128, 2(term hi/lo), 2(slot), ncols] fp8.

    term 0/1 = hi/lo of the value; slot i pairs dim rows (c*256 + i*128 + p)
    for the DoubleRow 256-deep contraction."""
    f8 = ml_dtypes.float8_e4m3
    hi = a.astype(f8)
    lo = (a - hi.astype(np.float32)).astype(f8)
    KC2 = dim // 256
    # [dim, ncols] -> [KC2, 2(slot), 128, ncols] -> [KC2, 128, 2slot, ncols]
    def arr(t):
        return t.reshape(KC2, 2, 128, ncols).transpose(0, 2, 1, 3)
    out = np.stack([arr(hi), arr(lo)], axis=2)  # [KC2, 128, term, slot, cols]
    return np.ascontiguousarray(out)


def prep_core_inputs(x, w_qkv, w_out, n, dim, hpc):
    """Host-side prep for ONE core (layout only: transpose/chunk/cast)."""
    inner = hpc * DH
    x = np.asarray(x, np.float32)
    w_qkv = np.asarray(w_qkv, np.float32)
    bf = ml_dtypes.bfloat16
    xT = np.ascontiguousarray(x.T) * np.float32(XS)
    return {
        "xt": _hilo_dr(xT, dim, n),
        "wq": _hilo_dr(w_qkv[:, 0:inner] * np.float32(WS), dim, inner),
        "wk": _hilo_dr(w_qkv[:, inner:2 * inner] * np.float32(WS), dim,
                       inner),
        "wv": _hilo_dr(w_qkv[:, 2 * inner:3 * inner] * np.float32(WS), dim,
                       inner),
        "wout": np.ascontiguousarray(
            np.asarray(w_out, np.float32).reshape(
                inner // 128, 128, dim) / np.float32(XS * WS)).astype(bf),
    }


def make_in_maps(x, w_qkv, w_out):
    """Shard full inputs into the 8 per-core input maps."""
    x = np.asarray(x, dtype=np.float32)
    w_qkv = np.asarray(w_qkv, dtype=np.float32)
    w_out = np.asarray(w_out, dtype=np.float32)
    qk = HEADS_FULL * DH
    in_maps = []
    for core in range(N_CORES):
        b, g = divmod(core, GROUPS)
        cols = ts(g, INNER_PC)
        wqkv_c = np.concatenate(
            [w_qkv[:, cols],
             w_qkv[:, qk + g * INNER_PC:qk + (g + 1) * INNER_PC],
             w_qkv[:, 2 * qk + g * INNER_PC:2 * qk + (g + 1) * INNER_PC]],
            axis=1)
        in_maps.append(prep_core_inputs(x[b], wqkv_c, w_out[cols, :],
                                        N_FULL, DIM_FULL, HPC))
    return in_maps


def kernel(x, w_qkv, w_out, b_out, trace=False):
    b_out = np.asarray(b_out, dtype=np.float32)
    in_maps = make_in_maps(x, w_qkv, w_out)
    nc = build_nc()
    res = bass_utils.run_bass_kernel_spmd(
        nc, in_maps, core_ids=list(range(N_CORES)), trace=trace)
    ys = [r["y"] for r in res.results]
    out = np.empty((B_FULL, N_FULL, DIM_FULL), dtype=np.float32)
    for b in range(B_FULL):
        out[b] = ys[GROUPS * b] + ys[GROUPS * b + 1] + b_out[None, :]
    if trace:
        kernel.last_result = res
    return out


# revision 58
# speedup vs baseline: 1.0715x; 1.0012x over previous
"""Multi-head attention (b=4, n=2048, dim=1024, 16 heads x 64) on 8 Trainium2
NeuronCores.

Sharding: data-parallel over batch (4) x tensor-parallel over head-groups (2).
Each core gets one batch element and 8 heads; the host sums the two
head-group partials per batch element and adds b_out.

Per-core pipeline (fp8 hi/lo DoubleRow projections, bf16 attention, fp32
accumulation):
  A:  x^T and the qkv weights arrive from the host as hi/lo fp8e4 pairs in
      DoubleRow slot layout (x*4, w*64 pre-scales keep the lo residuals
      above e4m3's subnormal floor; compensated in the exp scale and w_out).
      Each projection accumulates 3 cross terms (xh*wh + xh*wl + xl*wh) per
      256-deep DoubleRow chunk-pair at 0.5 cycles/row -- 25% fewer PE cycles
      than bf16, and ~2x less quantization error.  qT / kT strips
      ([inner, n], bf16) and v (natural [n, inner] + a ones column per head,
      bf16) stay SBUF-resident.  Phase A's projection strips are WOVEN into
      phase B's step stream (each strip emitted just before its first
      consuming attention step), so the PE fills the startup DMA window with
      projection work instead of idling.
  B:  per (i-block, head): S^T j-tiles = matmul(lhsT=k^T_h j-block,
      rhs=q^T_h i-block); exp on ScalarE (1/sqrt(dh) folded into the
      activation scale) writes bf16 pexp tiles which stay resident for the
      whole head.  PV is FLIPPED: po[i-128, dh+1] psum accumulates
      matmul(lhsT=pexp_jt[:, i-chunk], rhs=v_jt[:, head cols]) over all 16
      j-tiles -- out free size is only dh+1=65, so PV costs ~2x fewer PE
      cycles than the [dh+1, i]-oriented version (cost model charges
      out-free-size per pass).  The last v column is ones, so po[:, dh] is
      the softmax denominator: DVE reciprocal + per-partition tensor_scalar
      multiply normalizes straight into a bf16 staging tile, PE transposes
      it (identity trick) to [dh, 128] psum, DVE copies into the oT strips.
      PV/normalize/transpose of head h-1 is interleaved into head h's score
      stream so each small psum pool (1 buf each) has time to drain.
  C:  y = O @ w_out via lhsT = oT strips (bf16), micro-woven (matmul at a
      time) into attention steps where the phase-A stream has nothing due;
      DVE drains the proj psums, y DMA'd out per 512-column chunk.
"""

from contextlib import ExitStack

import numpy as np
import ml_dtypes

import concourse.mybir as mybir
import concourse.tile as tile
from concourse import bacc, bass_utils, masks
from concourse import dve_ops as _dvo
from concourse.dve_spec import (
    C0 as _C0, C1 as _C1, C2 as _C2, One as _One,
    AluOp as _AluOp, Bin as _Bin, Spec as _Spec, Src0 as _Src0,
)


def _sq(x):
    return _Bin(_AluOp.MULTIPLY, x, x)


def _exp2a_ref(in0, in1, s0, s1, imm2):
    u = in0.astype(np.float32) * np.float32(s0)
    p = (np.float32(s1) * u + np.float32(imm2)) * u + np.float32(1.0)
    p = (p * p).astype(np.float32)
    return (p * p).astype(np.float32)


def _exp2b_ref(in0, in1, s0, s1, imm2):
    x = in0.astype(np.float32)
    for _ in range(4):
        x = (x * x).astype(np.float32)
    return x


# exp(logit) = 2^(64u) = (quad(u))^64 with u = logit*log2e/64: op A scales,
# evaluates the minimax quadratic for 2^u (|u| <= ~0.2) and squares twice;
# op B squares four more times.  Max rel err ~0.55% for |logit| <= 8 -- used
# on a fraction of the softmax tiles to offload ScalarE.
_u_ = _Src0 * _C0
_p_ = (_C1 * _u_ + _C2) * _u_ + _One
EXP2A = _dvo.DveOp("EXP2_LADDER_A", _Spec(body=_sq(_sq(_p_)),
                                          reference=_exp2a_ref),
                   subdim=False, uops_sha={"v3": "c3940ebb62d9d92f"})
EXP2B = _dvo.DveOp("EXP2_LADDER_B", _Spec(body=_sq(_sq(_sq(_sq(_Src0)))),
                                          reference=_exp2b_ref),
                   subdim=False, uops_sha={"v3": "6d6edb7498c4a68d"})
if "EXP2_LADDER_A" not in _dvo._SUB_OPCODE_FOR_NAME:
    _dvo._SUB_OPCODE_FOR_NAME["EXP2_LADDER_A"] = 17
    _dvo._SUB_OPCODE_FOR_NAME["EXP2_LADDER_B"] = 18
    _dvo.OPS.append(EXP2A)
    _dvo.OPS.append(EXP2B)
    _dvo.CUSTOM_DVE_SPECS["EXP2_LADDER_A"] = EXP2A.spec
    _dvo.CUSTOM_DVE_SPECS["EXP2_LADDER_B"] = EXP2B.spec

# minimax quadratic coefficients for 2^u, |u| <= 8*log2e/64
EXP2_C1 = 0.6944773368902336
EXP2_C2 = 0.2402110103092763

# offload every k-th softmax tile's exp to the DVE ladder (0 = all ScalarE)
import os as _os
DVE_EVERY_DEFAULT = int(_os.environ.get("DVE_EVERY", "0"))

F32 = mybir.dt.float32
BF16 = mybir.dt.bfloat16
F8 = mybir.dt.float8e4
DR = mybir.MatmulPerfMode.DoubleRow
AF = mybir.ActivationFunctionType
ALU = mybir.AluOpType

# (x-term, w-term) index pairs for the hi/lo fp8 product expansion
# x·w ~= xh·wh + xh·wl + xl·wh  (xl·wl ~ 0.1% of x·w, dropped)
HL_TERMS = ((0, 0), (0, 1), (1, 0))

# Pre-quantization scales so the lo residuals stay above e4m3's subnormal
# floor (2^-9): x ~ N(0,1) -> x*XS; w ~ N(0, 1/dim) -> w*WS.  Compensated in
# the exp activation scale (scores carry (XS*WS)^2) and in w_out (o carries
# XS*WS).
XS, WS = 4.0, 64.0

# Full-problem constants (hardcoded per the harness contract).
B_FULL, N_FULL, DIM_FULL = 4, 2048, 1024
HEADS_FULL, DH = 16, 64
N_CORES = 8
GROUPS = 2                       # head-group (tensor-parallel) factor
HPC = HEADS_FULL // GROUPS       # heads per core = 8
INNER_PC = HPC * DH              # per-core inner dim = 512


def ts(i, size):
    return slice(i * size, (i + 1) * size)


def emit_core_kernel(nc, tc, xt, wq, wk, wv, wout, y, *, n, dim, hpc,
                     ib=1024, ahead=0, proj_every=2, dve_every=0):
    dh = DH
    inner = hpc * dh
    KC2 = dim // 256         # DoubleRow contraction chunk-pairs
    S = inner // 128         # 128-row strips of the inner dim
    JT = n // 128            # key/value j-tiles
    NB = n // 512            # 512-wide n-chunks in phase A
    ib = min(ib, n)
    n_ibx = n // ib
    itpb = ib // 128         # i-128 chunks per i-block
    cpb = ib // 512          # 512-chunks per i-block
    scale = float(1.0 / np.sqrt(dh) / (XS * WS) ** 2)
    fc = min(512, dim)
    assert n % 512 == 0 and dim % 128 == 0 and ib % 512 == 0

    stack = ExitStack()
    with stack:
        persist = stack.enter_context(tc.tile_pool(name="persist", bufs=1))
        w_pool = stack.enter_context(tc.tile_pool(name="a_w", bufs=1))
        xts_pool = stack.enter_context(
            tc.tile_pool(name="a_xts", bufs=KC2 * NB))
        psA_pool = stack.enter_context(
            tc.tile_pool(name="a_ps", bufs=2, space="PSUM"))

        qT = [persist.tile([128, n], BF16, name="qT", tag=f"qT{s}")
              for s in range(S)]
        kT = [persist.tile([128, n], BF16, name="kT", tag=f"kT{s}")
              for s in range(S)]
        v_sb = [persist.tile([128, hpc * (dh + 1)], BF16, name="vt",
                             tag=f"v{j}") for j in range(JT)]
        oT = [persist.tile([128, n], BF16, name="oT", tag=f"oT{s}")
              for s in range(S)]
        ident = persist.tile([128, 128], BF16, name="ident")

        # weight DMAs: wk first (first strips), wv/wq next, wout last (only
        # needed by the first proj group, far in).
        # All projection operands are hi/lo fp8 pairs in DoubleRow layout:
        # x8 [KC2, 128, term, slot, n], w8 [KC2, 128, term, slot, inner].
        wq_sb, wk_sb, wv_sb, wout_sb = [], [], [], []
        xts_tiles = {}

        def xts(c, nb, eng=None):
            t = xts_tiles.get((c, nb))
            if t is None:
                t = xts_pool.tile([128, 2, 2, 512], F8, name="xts")
                if eng is None:
                    eng = nc.gpsimd if (c % 2) == 1 else nc.sync
                eng.dma_start(t, xt[c][:, :, :, ts(nb, 512)])
                xts_tiles[(c, nb)] = t
            return t

        # DMA priority order: strip-0 columns of wk (+x chunk 0) feed the
        # very first projection strips; then strip-0 of wq (+x chunk 1),
        # then wv (first v tile ~10 steps in), then the remaining w columns,
        # remaining x chunks, and wout (first consumed much later) last.
        for nm, lst in (("wk", wk_sb), ("wq", wq_sb)):
            if inner > 128:
                for c in range(KC2):
                    lst.append(w_pool.tile([128, 2, 2, inner - 128], F8,
                                           name="wt", tag=f"{nm}{c}"))
        for c in range(KC2):
            wv_sb.append(w_pool.tile([128, 2, 2, inner], F8, name="wt",
                                     tag=f"wv{c}"))
        # strip-0 weight columns for all chunk-pairs land as ONE strided DMA
        # each (the per-DMA queue cadence, not transfer size, paces startup)
        wks0 = w_pool.tile([128, KC2, 2, 2, 128], F8, name="wks0")
        for c in range(KC2):
            nc.sync.dma_start(wks0[:, c], wk[c][:, :, :, 0:128])
            xts(c, 0)
        wqs0 = w_pool.tile([128, KC2, 2, 2, 128], F8, name="wqs0")
        for c in range(KC2):
            (nc.sync if c % 2 == 0 else nc.gpsimd).dma_start(
                wqs0[:, c], wq[c][:, :, :, 0:128])
            if NB > 1:
                xts(c, 1)
        for c in range(KC2):
            nc.sync.dma_start(wv_sb[c], wv[c])
        for c in range(KC2):
            if inner > 128:
                nc.sync.dma_start(wk_sb[c], wk[c][:, :, :, 128:inner])
                nc.sync.dma_start(wq_sb[c], wq[c][:, :, :, 128:inner])
        for nb in range(2, NB):
            for c in range(KC2):
                xts(c, nb)
        for t in range(S):
            wo = persist.tile([128, dim], BF16, name="wo", tag=f"wo{t}")
            nc.sync.dma_start(wo, wout[t])
            wout_sb.append(wo)
        # deferred Pool work: emitted after the DMA issues so the Pool
        # sequencer (25ns/DMA vs SP's 565ns) is free at t=0 for the
        # startup-critical x-chunk transfers
        for j in range(JT):
            nc.gpsimd.memset(
                v_sb[j].rearrange("p (h c) -> p h c", c=dh + 1)
                [:, :, dh:dh + 1], 1.0)
        masks.make_identity(nc, ident)

        # ---- phase A ops (generators, woven into phase B at matmul
        #      granularity) ----
        def qk_strip_gen(w_sb, dst, t, nb):
            s0 = wks0 if w_sb is wk_sb else wqs0
            ps = psA_pool.tile([128, 512], F32, name="psA")
            nmm = KC2 * len(HL_TERMS)
            i = 0
            for c in range(KC2):
                for ta, tb in HL_TERMS:
                    lhsT = (s0[:, c, tb] if t == 0
                            else w_sb[c][:, tb, :, ts(t - 1, 128)])
                    nc.tensor.matmul(ps, lhsT, xts(c, nb)[:, ta],
                                     start=(i == 0), stop=(i == nmm - 1),
                                     perf_mode=DR)
                    i += 1
                    yield
            nc.vector.tensor_copy(dst[t][:, ts(nb, 512)], ps)

        def v_tile_gen(it):
            ps = psA_pool.tile([128, inner], F32, name="psA")
            nmm = KC2 * len(HL_TERMS)
            i = 0
            for c in range(KC2):
                for ta, tb in HL_TERMS:
                    nc.tensor.matmul(
                        ps, xts(c, it // 4)[:, ta, :, ts(it % 4, 128)],
                        wv_sb[c][:, tb],
                        start=(i == 0), stop=(i == nmm - 1), perf_mode=DR)
                    i += 1
                    yield
            nc.vector.tensor_copy(
                v_sb[it].rearrange("p (h c) -> p h c", c=dh + 1)[:, :, 0:dh],
                ps.rearrange("p (h d) -> p h d", d=dh))

        # need-index: flat B step index (ibx*hpc*JT + h*JT + jt) of the first
        # consumer of each A op.
        a_ops = []
        for s in range(S):
            for nb in range(NB):
                a_ops.append((2 * s * JT + 4 * nb, 0,
                              lambda s=s, nb=nb: qk_strip_gen(
                                  wk_sb, kT, s, nb)))
        for s in range(S):
            for ibx in range(n_ibx):
                for nb in range(ibx * ib // 512, (ibx + 1) * ib // 512):
                    a_ops.append((ibx * hpc * JT + 2 * s * JT, 1,
                                  lambda s=s, nb=nb: qk_strip_gen(
                                      wq_sb, qT, s, nb)))
        for it in range(JT):
            # v tiles are first read by PV of head 0, which is interleaved
            # into the SECOND half of head 1's score stream -- spread their
            # emission across heads 0 and 1 instead of bursting at head 0
            a_ops.append((1 + it * 3 // 2, 2, lambda it=it: v_tile_gen(it)))
        a_ops.sort(key=lambda x: (x[0], x[1]))
        a_state = {"ptr": 0, "gen": None}

        def a_step():
            """Advance the A stream one micro-op; False when exhausted."""
            while True:
                if a_state["gen"] is None:
                    if a_state["ptr"] >= len(a_ops):
                        return False
                    a_state["gen"] = a_ops[a_state["ptr"]][2]()
                try:
                    next(a_state["gen"])
                    return True
                except StopIteration:
                    a_state["gen"] = None
                    a_state["ptr"] += 1

        def pump_a(limit, budget=None):
            n_done = 0
            while True:
                if budget is not None and n_done >= budget:
                    return
                if a_state["ptr"] >= len(a_ops):
                    return
                if budget is None and a_ops[a_state["ptr"]][0] > limit:
                    return
                if not a_step():
                    return
                n_done += 1

        # ---- phase B/C ----
        with (
            tc.tile_pool(name="b_psS", bufs=2, space="PSUM") as psS_pool,
            # po ([128, dh+1] f32) and trp ([dh, 128] bf16) alternate through
            # the same two bank-sized slots: po(ci+1) waits only on po(ci)'s
            # DVE readers, trp(ci+1) only on trp(ci)'s DVE copy
            tc.tile_pool(name="b_psO", bufs=2, space="PSUM") as psO_pool,
            tc.tile_pool(name="b_pexp", bufs=2 * JT + 2) as pexp_pool,
            tc.tile_pool(name="b_ost", bufs=3) as ost_pool,
            tc.tile_pool(name="b_rec", bufs=3) as rec_pool,
            tc.tile_pool(name="c_y", bufs=2) as y_pool,
        ):
            ysb_open = {}

            def emit_proj_group(it, cc, final=False):
                if cc == 0:
                    ysb_open[it] = y_pool.tile([128, dim], F32, name="ysb")
                ysb = ysb_open[it]
                if final:
                    # alternate psA with the (by now idle) psS slots so the
                    # last i-block's groups pipeline 4 deep
                    if (it * (dim // fc) + cc) % 2 == 0:
                        ps = psS_pool.tile([128, fc], F32, name="psS")
                    else:
                        ps = psA_pool.tile([128, fc], F32, name="psA")
                else:
                    ps = psA_pool.tile([128, fc], F32, name="psA")
                for t in range(S):
                    nc.tensor.matmul(
                        ps, oT[t][:, ts(it, 128)], wout_sb[t][:, ts(cc, fc)],
                        start=(t == 0), stop=(t == S - 1))
                nc.vector.tensor_copy(ysb[:, ts(cc, fc)], ps)
                nc.sync.dma_start(y[ts(it, 128), ts(cc, fc)],
                                  ysb[:, ts(cc, fc)])
                if cc == dim // fc - 1:
                    del ysb_open[it]

            proj_due = []
            proj_state = {"gen": None}

            def proj_group_gen(it, cc):
                if cc == 0:
                    ysb_open[it] = y_pool.tile([128, dim], F32, name="ysb")
                ysb = ysb_open[it]
                ps = psA_pool.tile([128, fc], F32, name="psA")
                for t in range(S):
                    nc.tensor.matmul(
                        ps, oT[t][:, ts(it, 128)], wout_sb[t][:, ts(cc, fc)],
                        start=(t == 0), stop=(t == S - 1))
                    yield
                nc.vector.tensor_copy(ysb[:, ts(cc, fc)], ps)
                nc.sync.dma_start(y[ts(it, 128), ts(cc, fc)],
                                  ysb[:, ts(cc, fc)])
                if cc == dim // fc - 1:
                    del ysb_open[it]

            def pump_proj(budget):
                n_done = 0
                while n_done < budget and (proj_state["gen"] or proj_due):
                    if proj_state["gen"] is None:
                        proj_state["gen"] = proj_group_gen(*proj_due.pop(0))
                    try:
                        next(proj_state["gen"])
                        n_done += 1
                    except StopIteration:
                        proj_state["gen"] = None

            def a_idle(idx):
                return (a_state["ptr"] >= len(a_ops)
                        and a_state["gen"] is None) or (
                    a_state["gen"] is None
                    and a_ops[a_state["ptr"]][0] > idx + ahead)

            def pv_gen(h, ibx, pexp_tiles):
                """Flipped PV + normalize + transpose for one (head, i-block).
                One yield per i-128 chunk; each chunk's transpose+copy are
                deferred to the NEXT step so the PE never waits on the
                freshly-issued DVE recip/normalize chain."""
                s_, r_ = divmod(h * dh, 128)
                vcol = slice(h * (dh + 1), (h + 1) * (dh + 1))

                def finish(ost, ci):
                    c0 = ibx * ib + ci * 128
                    trp = psO_pool.tile([dh, 128], BF16, name="trp", tag="pot")
                    nc.tensor.transpose(trp, ost, ident)
                    nc.vector.tensor_copy(oT[s_][r_:r_ + dh, c0:c0 + 128],
                                          trp)

                prev = None
                for ci in range(itpb):
                    if prev is not None:
                        finish(*prev)
                    po = psO_pool.tile([128, dh + 1], F32, name="po", tag="pot")
                    for jt in range(JT):
                        nc.tensor.matmul(
                            po, pexp_tiles[jt][:, ts(ci, 128)],
                            v_sb[jt][:, vcol],
                            start=(jt == 0), stop=(jt == JT - 1))
                    recip = rec_pool.tile([128, 1], F32, name="recip")
                    nc.vector.reciprocal(recip, po[:, dh:dh + 1])
                    ost = ost_pool.tile([128, dh], BF16, name="ost")
                    nc.vector.tensor_scalar_mul(ost, po[:, 0:dh], recip)
                    prev = (ost, ci)
                    yield
                finish(*prev)

            pv_state = {"gen": None, "done": 0, "block_done": None}
            _DONE = object()

            def pump_pv(want):
                st = pv_state
                while st["gen"] is not None and st["done"] < want:
                    if next(st["gen"], _DONE) is _DONE:
                        st["gen"] = None
                        if st["block_done"] is not None:
                            # last head of block finished: its out-projection
                            # groups may now be emitted (all oT writes for the
                            # block precede them in program order)
                            bx = st["block_done"]
                            proj_due.extend(
                                (it, cc)
                                for it in range(bx * itpb, (bx + 1) * itpb)
                                for cc in range(dim // fc))
                        break
                    st["done"] += 1

            for ibx in range(n_ibx):
                for h in range(hpc):
                    s_, r_ = divmod(h * dh, 128)
                    kTh = kT[s_][r_:r_ + dh, :]
                    qTh = qT[s_][r_:r_ + dh, :]
                    pexp_tiles = []
                    for jt in range(JT):
                        idx = ibx * hpc * JT + h * JT + jt
                        pump_a(idx)
                        psS = psS_pool.tile([128, ib], F32, name="psS")
                        for cc in range(cpb):
                            nc.tensor.matmul(
                                psS[:, ts(cc, 512)], kTh[:, ts(jt, 128)],
                                qTh[:, ibx * ib + cc * 512:
                                    ibx * ib + (cc + 1) * 512],
                                start=True, stop=True)
                        pexp = pexp_pool.tile([128, ib], BF16, name="pexp")
                        lastpair = (ibx == n_ibx - 1 and h == hpc - 1)
                        use_dve = (dve_every and
                                   jt % dve_every == dve_every - 1) or (
                                   dve_last and lastpair and jt < JT // 2
                                   and jt % dve_last == 1)
                        if use_dve:
                            nc.vector._custom_dve(
                                EXP2A, out=psS, in0=psS,
                                s0=float(scale * np.log2(np.e) / 64.0),
                                s1=EXP2_C1, imm2=EXP2_C2)
                            nc.vector._custom_dve(EXP2B, out=pexp, in0=psS)
                        else:
                            nc.scalar.activation(pexp, psS, AF.Exp,
                                                 scale=scale)
                        pexp_tiles.append(pexp)
                        # interleave PV chunks of the previous head into the
                        # SECOND half of this head's score stream (the v tiles
                        # PV reads are only projected by then, and the po/trp
                        # psum slots get time to drain between chunks)
                        st = 3 * JT // 8
                        pump_pv(max(0, (jt + 1 - st) * itpb // (JT - st)))
                        pump_a(idx + ahead)
                        if a_idle(idx):
                            pump_proj(proj_every)
                    # drain any PV leftovers of the previous head
                    pump_pv(10 ** 9)
                    last = (h == hpc - 1 and ibx == n_ibx - 1)
                    pv_state = {"gen": pv_gen(h, ibx, pexp_tiles), "done": 0,
                                "block_done":
                                    ibx if (h == hpc - 1 and not last)
                                    else None}
            # tail: interleave the last head's PV chunks with that block's
            # projection groups (4-deep psum rotation) so the drains pipeline
            last_bx = n_ibx - 1
            for ci in range(itpb):
                pump_pv(ci + 1)
                if ci >= 1:
                    # chunk ci-1's transpose+copy were emitted during step ci
                    for cc in range(dim // fc):
                        emit_proj_group(last_bx * itpb + ci - 1, cc,
                                        final=True)
                pump_proj(4)
            pump_pv(10 ** 9)
            for cc in range(dim // fc):
                emit_proj_group(last_bx * itpb + itpb - 1, cc, final=True)
            pump_proj(10 ** 9)
            pump_a(10 ** 9)


_BUILD_CACHE = {}


def build_nc(n=N_FULL, dim=DIM_FULL, hpc=HPC, ib=1024, ahead=0,
             proj_every=2, dve_every=DVE_EVERY_DEFAULT):
    key = (n, dim, hpc, ib, ahead, proj_every, dve_every)
    if key in _BUILD_CACHE:
        return _BUILD_CACHE[key]
    inner = hpc * DH
    KC2 = dim // 256
    S = inner // 128
    nc = bacc.Bacc("TRN2", target_bir_lowering=False, debug=False)
    xt = nc.dram_tensor("xt", [KC2, 128, 2, 2, n], F8,
                        kind="ExternalInput").ap()
    wq = nc.dram_tensor("wq", [KC2, 128, 2, 2, inner], F8,
                        kind="ExternalInput").ap()
    wk = nc.dram_tensor("wk", [KC2, 128, 2, 2, inner], F8,
                        kind="ExternalInput").ap()
    wv = nc.dram_tensor("wv", [KC2, 128, 2, 2, inner], F8,
                        kind="ExternalInput").ap()
    wout = nc.dram_tensor("wout", [S, 128, dim], BF16,
                          kind="ExternalInput").ap()
    y = nc.dram_tensor("y", [n, dim], F32, kind="ExternalOutput").ap()
    with tile.TileContext(nc) as tc:
        with nc.allow_low_precision(
                reason="bf16 attention operands; fp32 accumulation"):
            emit_core_kernel(nc, tc, xt, wq, wk, wv, wout, y, n=n, dim=dim,
                             hpc=hpc, ib=ib, ahead=ahead,
                             proj_every=proj_every, dve_every=dve_every)
    nc.compile()
    _BUILD_CACHE[key] = nc
    return nc


def _hilo_dr(a, dim, ncols):
    """[dim, ncols] f32 -> [dim/256, 128, 2(term hi/lo), 2(slot), ncols] fp8.

    term 0/1 = hi/lo of the value; slot i pairs dim rows (c*256 + i*128 + p)
    for the DoubleRow 256-deep contraction."""
    f8 = ml_dtypes.float8_e4m3
    hi = a.astype(f8)
    lo = (a - hi.astype(np.float32)).astype(f8)
    KC2 = dim // 256
    # [dim, ncols] -> [KC2, 2(slot), 128, ncols] -> [KC2, 128, 2slot, ncols]
    def arr(t):
        return t.reshape(KC2, 2, 128, ncols).transpose(0, 2, 1, 3)
    out = np.stack([arr(hi), arr(lo)], axis=2)  # [KC2, 128, term, slot, cols]
    return np.ascontiguousarray(out)


def prep_core_inputs(x, w_qkv, w_out, n, dim, hpc):
    """Host-side prep for ONE core (layout only: transpose/chunk/cast)."""
    inner = hpc * DH
    x = np.asarray(x, np.float32)
    w_qkv = np.asarray(w_qkv, np.float32)
    bf = ml_dtypes.bfloat16
    xT = np.ascontiguousarray(x.T) * np.float32(XS)
    return {
        "xt": _hilo_dr(xT, dim, n),
        "wq": _hilo_dr(w_qkv[:, 0:inner] * np.float32(WS), dim, inner),
        "wk": _hilo_dr(w_qkv[:, inner:2 * inner] * np.float32(WS), dim,
                       inner),
        "wv": _hilo_dr(w_qkv[:, 2 * inner:3 * inner] * np.float32(WS), dim,
                       inner),
        "wout": np.ascontiguousarray(
            np.asarray(w_out, np.float32).reshape(
                inner // 128, 128, dim) / np.float32(XS * WS)).astype(bf),
    }


def make_in_maps(x, w_qkv, w_out):
    """Shard full inputs into the 8 per-core input maps."""
    x = np.asarray(x, dtype=np.float32)
    w_qkv = np.asarray(w_qkv, dtype=np.float32)
    w_out = np.asarray(w_out, dtype=np.float32)
    qk = HEADS_FULL * DH
    in_maps = []
    for core in range(N_CORES):
        b, g = divmod(core, GROUPS)
        cols = ts(g, INNER_PC)
        wqkv_c = np.concatenate(
            [w_qkv[:, cols],
             w_qkv[:, qk + g * INNER_PC:qk + (g + 1) * INNER_PC],
             w_qkv[:, 2 * qk + g * INNER_PC:2 * qk + (g + 1) * INNER_PC]],
            axis=1)
        in_maps.append(prep_core_inputs(x[b], wqkv_c, w_out[cols, :],
                                        N_FULL, DIM_FULL, HPC))
    return in_maps


def kernel(x, w_qkv, w_out, b_out, trace=False):
    b_out = np.asarray(b_out, dtype=np.float32)
    in_maps = make_in_maps(x, w_qkv, w_out)
    nc = build_nc()
    res = bass_utils.run_bass_kernel_spmd(
        nc, in_maps, core_ids=list(range(N_CORES)), trace=trace)
    ys = [r["y"] for r in res.results]
    out = np.empty((B_FULL, N_FULL, DIM_FULL), dtype=np.float32)
    for b in range(B_FULL):
        out[b] = ys[GROUPS * b] + ys[GROUPS * b + 1] + b_out[None, :]
    if trace:
        kernel.last_result = res
    return out
